# revision 2
# baseline (speedup 1.0000x reference)
"""GQA (16 q-heads / 4 kv-heads, D=128, S=2048, E=2048, B=2) on 8 trn2 cores.

Sharding: core = 4*b + g  (b in {0,1} batch, g in {0..3} kv-head group).
Each core computes its batch's 4 query heads (one kv group) end-to-end and
the host sums the 4 partial o_proj outputs per batch.

v3 (single interleaved program, engines balanced, DMA-consolidated):
  - Blocked DRAM layouts + AP.transpose give one DMA per weight tensor and
    4 DMAs per x position-chunk (~65 DMAs total vs ~250).
  - Phase A per 512-position chunk: K proj+RoPE, Q(h0) proj+RoPE, V proj +
    PE-transpose to natural bf16, Q(h1..h3) proj+RoPE.  RoPE rotate-half is
    a signed-permutation matmul on PE (no SBUF swap DMAs); cos/sin mults
    split DVE/DVE, add on Pool.
  - Attention in four 512-wide query chunks: scoresT per sk-tile in f32r,
    Exp on Act straight from PSUM to bf16 SBUF, bf16 AV matmuls (V-natural
    stationary), softmax denominator via two bf16 chain-adds (DVE + Pool),
    ones-matmul partition reduce, reciprocal, Pool partition_broadcast,
    DVE normalize multiply into f32r ot.
  - o_proj of chunk c interleaved into attention of chunk c+1; stores
    staged through SBUF [128,1024] tiles via DVE/Act copies.
"""

import numpy as np
import ml_dtypes

import concourse.bass as bass
import concourse.bacc as bacc
import concourse.mybir as mybir
import concourse.tile as tile
from concourse.bass_utils import run_bass_kernel_spmd

B, S, E = 2, 2048, 2048
H, HKV, D = 16, 4, 128
G = H // HKV          # 4 query heads per kv group
GD = G * D            # 512 channels per group
NCORES = 8
SCALE = 1.0 / float(np.sqrt(D))
ROPE_BASE = 10000.0
AX = 16.0             # fp8 plane scale for x
AW = 64.0             # fp8 plane scale for wq/wk/wv/wo
PSC = AX * AW         # q/k/v come out scaled by PSC
SCALE_EFF = SCALE / (PSC * PSC)   # folds the q*k scale into exp
AO = 16.0             # fp8 plane scale for the normalized attention output
# the softmax reduce uses (PSC/AO)-valued "ones", so ot = AO * attn_out and
# the o_proj result comes out scaled by AO*AW = PSC; the host divides once.
RED = PSC / AO

NE = E // 128         # 16 e-blocks (contraction for projections)
NC4 = S // 512        # 4 position chunks of 512 (projection granularity)
NST = S // 128        # 16 sk-tiles of 128
CHS = [1024, 512, 512]          # attention query-chunk widths
COFF = [0, 1024, 1536]          # their offsets
CHMAX = 1024

F32 = mybir.dt.float32
F32R = mybir.dt.float32r
BF16 = mybir.dt.bfloat16
FP8 = mybir.dt.float8e4
DR = mybir.MatmulPerfMode.DoubleRow
AF = mybir.ActivationFunctionType
OP = mybir.AluOpType


def _r(ap):
    return ap.bitcast(F32R)


def _emit(nc, tc, xh, xl, wqh, wql, wkh, wkl, wvh, wvl, woh, wol, cosT, sinT, rotP, ident, onesb, out):
    from contextlib import ExitStack
    es = ExitStack()
    with es:
        cpool = es.enter_context(tc.tile_pool(name="const", bufs=1))
        wopool = es.enter_context(tc.tile_pool(name="wo", bufs=2))
        xpool = es.enter_context(tc.tile_pool(name="xs", bufs=16))
        rpool = es.enter_context(tc.tile_pool(name="rope", bufs=2))
        etpool = es.enter_context(tc.tile_pool(name="et", bufs=6))
        bcspool = es.enter_context(tc.tile_pool(name="bcs", bufs=1))
        dnpool = es.enter_context(tc.tile_pool(name="dn", bufs=2))
        rcpool = es.enter_context(tc.tile_pool(name="rc", bufs=1))
        otpool = es.enter_context(tc.tile_pool(name="ot", bufs=2))
        ostgpool = es.enter_context(tc.tile_pool(name="ostg", bufs=3))
        vtpool = es.enter_context(tc.tile_pool(name="vt", bufs=2))
        pssc = es.enter_context(
            tc.tile_pool(name="pssc", bufs=2, space=bass.MemorySpace.PSUM))
        psav = es.enter_context(
            tc.tile_pool(name="psav", bufs=1, space=bass.MemorySpace.PSUM))
        psmx = es.enter_context(
            tc.tile_pool(name="psmx", bufs=2, space=bass.MemorySpace.PSUM))

        # ---- small constants (rp needed first; the rest load after the
        # first x chunk, off the critical path to the first K matmul) ----
        id_sb = cpool.tile([128, 128], F32, tag="id")
        ones_sb = cpool.tile([128, 128], BF16, tag="ones")
        rp_sb = cpool.tile([128, 128], BF16, tag="rp")
        cos_sb = cpool.tile([D, S], BF16, tag="cos")
        sin_sb = cpool.tile([D, S], BF16, tag="sin")
        nc.sync.dma_start(out=rp_sb[:], in_=rotP.ap())

        # ---- consolidated weight loads (fp8 hi/lo double-quant planes);
        # wq/wv DMAs are issued after the first x chunk so the critical path
        # to the first K matmul is short
        wk_t = [cpool.tile([128, NE, D], FP8, tag=f"wkt{i}", name=f"wkt{i}")
                for i in range(2)]
        for i, t in enumerate((wkh, wkl)):
            nc.sync.dma_start(out=wk_t[i][:], in_=t.ap().transpose([1, 0, 2]))
        wq_t = [cpool.tile([128, NE, GD], FP8, tag=f"wqt{i}", name=f"wqt{i}")
                for i in range(2)]
        wv_t = [cpool.tile([128, NE, D], FP8, tag=f"wvt{i}", name=f"wvt{i}")
                for i in range(2)]

        # ---- persistent activations (bf16: same PE rate, half SBUF) ----
        kt = cpool.tile([D, S], BF16, tag="kt")
        qt = [cpool.tile([D, S], BF16, tag=f"qt{h}", name=f"qt{h}")
              for h in range(G)]
        vn = cpool.tile([128, NST, D], BF16, tag="vn")

        def rope(dst, ps, sl, in_attn=False):
            # rotate_half as a signed-permutation matmul; then
            # dst = q*cos + rot(q)*sin.  During phase A the qraw copy runs on
            # the idle Act engine and rot borrows an idle scores-pool PSUM
            # slot; inside attention windows both would collide with exp /
            # scores, so qraw moves to DVE and rot stays in psmx.
            qraw = rpool.tile([128, 512], BF16, tag="qraw")
            tmc = rpool.tile([128, 512], BF16, tag="tmc")
            t2 = rpool.tile([128, 512], BF16, tag="t2")
            if in_attn:
                nc.vector.tensor_copy(qraw[:], ps[:])
                rot = psmx.tile([128, 512], F32, tag="mx", name="rot")
            else:
                nc.scalar.copy(qraw[:], ps[:])
                rot = pssc.tile([128, 512], F32, tag="sc", name="rot")
            nc.tensor.matmul(rot[:], rp_sb[:], qraw[:], start=True, stop=True)
            nc.gpsimd.tensor_tensor(tmc[:], qraw[:], cos_sb[:, sl], OP.mult)
            nc.vector.tensor_tensor(t2[:], rot[:], sin_sb[:, sl], OP.mult)
            nc.vector.tensor_tensor(dst, tmc[:], t2[:], OP.add)

        def load_x(c4):
            sl = slice(c4 * 512, (c4 + 1) * 512)
            xsl = []
            for i, t in enumerate((xh, xl)):
                tiles = [xpool.tile([128, 8, 512], FP8, tag="xs",
                                    name=f"xs{c4}_{i}_{jj}")
                         for jj in range(2)]
                for jj in range(2):
                    nc.sync.dma_start(
                        out=tiles[jj][:],
                        in_=t.ap()[jj * 8:(jj + 1) * 8, :, sl]
                            .transpose([1, 0, 2]))
                xsl.append(tiles)
            return xsl

        PLANES = ((0, 0), (0, 1), (1, 0))   # (w_plane, x_plane): HH, HL, LH

        def proj_mms(ps, wt, cslice, xsl):
            mms = []
            for i, (wi, xi) in enumerate(PLANES):
                for p in range(NE // 2):
                    mms.append((wt[wi][:, 2 * p:2 * p + 2, cslice],
                                xsl[xi][p // 4][:, 2 * (p % 4):2 * (p % 4) + 2, :]))
            return mms

        def proj(wt, cslice, xsl):
            ps = psmx.tile([128, 512], F32, tag="mx", name="ps")
            mms = proj_mms(ps, wt, cslice, xsl)
            for i, (wa, xa) in enumerate(mms):
                nc.tensor.matmul(ps[:], wa, xa, perf_mode=DR,
                                 start=(i == 0), stop=(i == len(mms) - 1))
            return ps

        def qproj(h, c4, xsl):
            sl = slice(c4 * 512, (c4 + 1) * 512)
            ps = proj(wq_t, slice(h * D, (h + 1) * D), xsl)
            rope(qt[h][:, sl], ps, sl)

        # ============ phase A: K/V for all chunks, Q for chunk 0 ============
        # Remaining Q projections are interleaved into the act-gated
        # attention windows below (x slices reloaded through the same pool).
        # DMA order is tuned so nothing stalls: wv right behind the first x
        # chunk, wq behind the second, and the Q(chunk0) pass deferred until
        # after chunk 1's K/V so wq has certainly landed.
        def kv_chunk(c4, xsl):
            sl = slice(c4 * 512, (c4 + 1) * 512)
            ps = proj(wk_t, slice(0, D), xsl)
            rope(kt[:, sl], ps, sl)
            ps = proj(wv_t, slice(0, D), xsl)
            vt = vtpool.tile([D, 512], F32, tag="vt")
            nc.scalar.copy(vt[:], ps[:])
            for tt in range(4):
                trp = psmx.tile([128, 128], F32, tag="mx", name="trp")
                nc.tensor.transpose(trp[:], vt[:, tt * 128:(tt + 1) * 128],
                                    id_sb[:])
                nc.vector.tensor_copy(vn[:, c4 * 4 + tt, :], trp[:])

        nc.sync.dma_start(out=id_sb[:], in_=ident.ap())
        nc.sync.dma_start(out=cos_sb[:], in_=cosT.ap())
        nc.sync.dma_start(out=sin_sb[:], in_=sinT.ap())
        nc.sync.dma_start(out=ones_sb[:], in_=onesb.ap())
        xsl0 = load_x(0)
        for i, t in enumerate((wvh, wvl)):
            nc.sync.dma_start(out=wv_t[i][:], in_=t.ap().transpose([1, 0, 2]))
        xsl1 = load_x(1)
        for i, t in enumerate((wqh, wql)):
            for jj in range(2):
                nc.sync.dma_start(
                    out=wq_t[i][:, jj * 8:(jj + 1) * 8, :],
                    in_=t.ap()[jj * 8:(jj + 1) * 8, :, :]
                        .transpose([1, 0, 2]))
        kv_chunk(0, xsl0)
        xsl2 = load_x(2)
        kv_chunk(1, xsl1)
        for h in range(G):
            qproj(h, 0, xsl0)
        xsl3 = load_x(3)
        kv_chunk(2, xsl2)
        for h in range(G):
            qproj(h, 1, xsl1)
        kv_chunk(3, xsl3)

        # ================= phases B+C: attention + o_proj =================
        # Deferred PE work (remaining Q projections, previous-chunk o_proj)
        # is queued as ~1-matmul units and drained a couple per t-iteration,
        # so the Act engine never sees a multi-microsecond scores gap.
        import collections
        fill = collections.deque()

        def drain(n):
            while n > 0 and fill:
                fill.popleft()()
                n -= 1

        def enqueue_qproj(h, c4, xsl):
            sl = slice(c4 * 512, (c4 + 1) * 512)
            state = {}
            cslice = slice(h * D, (h + 1) * D)
            nmm = 3 * (NE // 2)

            def mk(j):
                def unit():
                    if j == 0:
                        state['ps'] = psmx.tile([128, 512], F32, tag="mx",
                                                name="ps")
                        state['mms'] = proj_mms(state['ps'], wq_t, cslice, xsl)
                    ps = state['ps']
                    for jj in (2 * j, 2 * j + 1):
                        wa, xa = state['mms'][jj]
                        nc.tensor.matmul(ps[:], wa, xa, perf_mode=DR,
                                         start=(jj == 0), stop=(jj == nmm - 1))
                return unit
            for j in range(nmm // 2):
                fill.append(mk(j))
            fill.append(lambda: rope(qt[h][:, sl], state['ps'], sl,
                                     in_attn=True))

        def attn_head(off, chw, h, ot_hi, ot_lo):
            """Scores/exp/AV/denominator/normalize for a chw-wide query chunk
            at offset off, head h.  Writes the AO-scaled normalized output
            into slice h of the chunk's fp8 hi/lo ot planes.

            Software-pipelined one t-tile deep: scores(t) + drained fill
            units run on PE before AV(t-1), so the Act engine's exp(t-1)
            has a full PE iteration of cover and AV never stalls on et."""
            nhf = chw // 512
            dn = dnpool.tile([128, chw], BF16, tag="dn")
            av = psav.tile([D, chw], F32, tag="av")

            def sc_exp(t):
                sc = pssc.tile([128, chw], F32, tag="sc")
                for hf in range(nhf):
                    qsl = slice(off + hf * 512, off + (hf + 1) * 512)
                    nc.tensor.matmul(sc[:, hf * 512:(hf + 1) * 512],
                                     kt[:, t * 128:(t + 1) * 128],
                                     qt[h][:, qsl], start=True, stop=True)
                et = etpool.tile([128, chw], BF16, tag="et")
                nc.scalar.activation(et[:], sc[:], AF.Exp, scale=SCALE_EFF)
                return et

            def av_mm(t, et):
                for hf in range(nhf):
                    nc.tensor.matmul(av[:, hf * 512:(hf + 1) * 512],
                                     vn[:, t, :],
                                     et[:, hf * 512:(hf + 1) * 512],
                                     start=(t == 0), stop=(t == NST - 1))

            et0 = None
            et_prev = sc_exp(0)
            for t in range(1, NST):
                et_t = sc_exp(t)
                drain(2)
                av_mm(t - 1, et_prev)
                j = t - 1
                if j == 0:
                    et0 = et_prev
                elif j == 1:
                    nc.vector.tensor_tensor(dn[:], et0[:], et_prev[:], OP.add)
                else:
                    nc.vector.tensor_tensor(dn[:], dn[:], et_prev[:], OP.add)
                et_prev = et_t
            drain(2)
            av_mm(NST - 1, et_prev)
            nc.vector.tensor_tensor(dn[:], dn[:], et_prev[:], OP.add)
            rc = rcpool.tile([1, chw], BF16, tag="rc")
            for hf in range(nhf):
                sm = psmx.tile([1, 512], F32, tag="mx", name="sm")
                nc.tensor.matmul(sm[:], ones_sb[:, 0:1],
                                 dn[:, hf * 512:(hf + 1) * 512],
                                 start=True, stop=True)
                with nc.allow_low_precision(reason="bf16 softmax denom recip"):
                    nc.vector.reciprocal(rc[:, hf * 512:(hf + 1) * 512], sm[:])
            bcs = bcspool.tile([128, chw], BF16, tag="bcs")
            nc.gpsimd.partition_broadcast(bcs[:], rc[:])
            otf = rpool.tile([D, chw], F32, tag="otf")
            nc.vector.tensor_tensor(otf[:], av[:], bcs[:], OP.mult)
            nc.scalar.copy(ot_hi[:, h, :], otf[:])
            nc.vector.tensor_tensor(ot_lo[:, h, :], otf[:], ot_hi[:, h, :],
                                    OP.subtract)

        def enqueue_oproj_pair(off, chw, eo0, oth, otl, ci, copy_eng,
                               tail=False):
            """Two adjacent eo column-groups (1024 cols of E) of o_proj for
            the chunk at offset off, via fp8 DoubleRow over (h-pair, plane):
            one [128,1024] staging tile per s-tile, one store each."""
            wots = []
            for eo in (eo0, eo0 + 1):
                wot = [wopool.tile([128, G, 512], FP8, tag=f"wo{i}",
                                   name=f"wo{ci}_{eo}_{i}") for i in range(2)]
                for i, t in enumerate((woh, wol)):
                    nc.sync.dma_start(
                        out=wot[i][:],
                        in_=t.ap()[:, :, eo * 512:(eo + 1) * 512]
                            .transpose([1, 0, 2]))
                wots.append(wot)
            for st in range(chw // 128):
                state = {}

                def mk(st, k):
                    def unit():
                        if k == 0:
                            state['ostg'] = ostgpool.tile(
                                [128, 1024], F32, tag="ostg",
                                name=f"ostg{ci}_{eo0}_{st}")
                        ostg = state['ostg']
                        op = psmx.tile([128, 512], F32, tag="mx",
                                       name="op")
                        ssl = slice(st * 128, (st + 1) * 128)
                        mms = []
                        for ota, wi in ((oth, 0), (oth, 1), (otl, 0)):
                            for i in range(2):
                                mms.append((ota[:, 2 * i:2 * i + 2, ssl],
                                            wots[k][wi][:, 2 * i:2 * i + 2, :]))
                        for i, (oa, wa) in enumerate(mms):
                            nc.tensor.matmul(op[:], oa, wa, perf_mode=DR,
                                             start=(i == 0),
                                             stop=(i == len(mms) - 1))
                        dst = ostg[:, k * 512:(k + 1) * 512]
                        eng = copy_eng
                        if eng == 'mix':
                            eng = 'act' if (st + k) % 2 else 'dve'
                        if eng == 'act':
                            nc.scalar.copy(dst, op[:])
                        else:
                            nc.vector.tensor_copy(dst, op[:])
                        if k == 1:
                            nc.sync.dma_start(
                                out=out.ap()[off + st * 128:
                                             off + (st + 1) * 128,
                                             eo0 * 512:(eo0 + 2) * 512],
                                in_=ostg[:])
                    return unit
                fill.append(mk(st, 0))
                fill.append(mk(st, 1))

        def ot_planes(ci):
            chw = CHS[ci]
            hi = otpool.tile([128, G, chw], FP8, tag="oth", name=f"oth{ci}")
            lo = otpool.tile([128, G, chw], FP8, tag="otl", name=f"otl{ci}")
            return hi, lo

        # B0 (512-wide): deferred Q projections for position chunks 1 and 2
        # drain into the PE slack of the attention loop (x stays resident).
        oth0, otl0 = ot_planes(0)
        for h in range(G):
            if h == 0:
                for hq in range(G):
                    enqueue_qproj(hq, 1, xsl1)
            elif h == 2:
                for hq in range(G):
                    enqueue_qproj(hq, 2, xsl2)
            attn_head(COFF[0], CHS[0], h, oth0, otl0)
            drain(8)
        drain(len(fill))
        # B1 (1024-wide): o_proj of chunk 0 + Q projections for chunk 3
        oth1, otl1 = ot_planes(1)
        for h in range(G):
            if h == 0:
                enqueue_oproj_pair(COFF[0], CHS[0], 0, oth0, otl0, 0, 'mix')
                for hq in range(G):
                    enqueue_qproj(hq, 3, xsl3)
                enqueue_oproj_pair(COFF[0], CHS[0], 2, oth0, otl0, 0, 'mix')
            attn_head(COFF[1], CHS[1], h, oth1, otl1)
            drain(8)
        drain(len(fill))
        # B2 (512-wide): o_proj of chunk 1, spread across all four heads
        oth2, otl2 = ot_planes(2)
        for h in range(G):
            if h == 0:
                enqueue_oproj_pair(COFF[1], CHS[1], 0, oth1, otl1, 1, 'mix')
            elif h == 2:
                enqueue_oproj_pair(COFF[1], CHS[1], 2, oth1, otl1, 1, 'mix')
            attn_head(COFF[2], CHS[2], h, oth2, otl2)
            drain(8)
        drain(len(fill))
        # tail: o_proj of chunk 2, staging copies on the now-idle Act engine
        enqueue_oproj_pair(COFF[2], CHS[2], 0, oth2, otl2, 2, 'mix', tail=True)
        enqueue_oproj_pair(COFF[2], CHS[2], 2, oth2, otl2, 2, 'mix', tail=True)
        drain(len(fill))


def _build():
    nc = bacc.Bacc("TRN2", target_bir_lowering=False, debug=False,
                   num_devices=NCORES)
    xh = nc.dram_tensor("xh", [NE, 128, S], FP8, kind="ExternalInput")
    xl = nc.dram_tensor("xl", [NE, 128, S], FP8, kind="ExternalInput")
    wqh = nc.dram_tensor("wqh", [NE, 128, GD], FP8, kind="ExternalInput")
    wql = nc.dram_tensor("wql", [NE, 128, GD], FP8, kind="ExternalInput")
    wkh = nc.dram_tensor("wkh", [NE, 128, D], FP8, kind="ExternalInput")
    wkl = nc.dram_tensor("wkl", [NE, 128, D], FP8, kind="ExternalInput")
    wvh = nc.dram_tensor("wvh", [NE, 128, D], FP8, kind="ExternalInput")
    wvl = nc.dram_tensor("wvl", [NE, 128, D], FP8, kind="ExternalInput")
    woh = nc.dram_tensor("woh", [G, 128, E], FP8, kind="ExternalInput")
    wol = nc.dram_tensor("wol", [G, 128, E], FP8, kind="ExternalInput")
    cosT = nc.dram_tensor("cosT", [D, S], BF16, kind="ExternalInput")
    sinT = nc.dram_tensor("sinT", [D, S], BF16, kind="ExternalInput")
    rotP = nc.dram_tensor("rotP", [128, 128], BF16, kind="ExternalInput")
    ident = nc.dram_tensor("ident", [128, 128], F32, kind="ExternalInput")
    onesb = nc.dram_tensor("onesb", [128, 128], BF16, kind="ExternalInput")
    out = nc.dram_tensor("out", [S, E], F32, kind="ExternalOutput")
    with tile.TileContext(nc) as tc:
        _emit(nc, tc, xh, xl, wqh, wql, wkh, wkl, wvh, wvl, woh, wol, cosT,
              sinT, rotP, ident, onesb, out)
    nc.compile()
    return nc


def _rope_tables():
    inv = 1.0 / (ROPE_BASE ** (np.arange(0, D, 2, dtype=np.float64) / D))
    t = np.arange(S, dtype=np.float64)
    freqs = t[:, None] * inv[None, :]                    # [S, D/2]
    emb = np.concatenate([freqs, freqs], axis=-1)        # [S, D]
    cosT = np.cos(emb).T.astype(ml_dtypes.bfloat16)      # [D, S]
    sinT = np.sin(emb).T.astype(ml_dtypes.bfloat16)
    return np.ascontiguousarray(cosT), np.ascontiguousarray(sinT)


def _rot_perm():
    # rot(q)[d] = -q[d+64] for d<64, +q[d-64] for d>=64, as a stationary
    # matmul operand: rot = P^T @ q with P[k, m] below.
    p = np.zeros((128, 128), dtype=ml_dtypes.bfloat16)
    for d in range(64):
        p[d + 64, d] = -1.0
        p[d, d + 64] = 1.0
    return p


_NC = None
LAST_RESULTS = None


def kernel(hidden_states, wq, wk, wv, wo):
    global _NC, LAST_RESULTS
    if _NC is None:
        _NC = _build()
    cosT, sinT = _rope_tables()
    ident = np.eye(128, dtype=np.float32)
    onesb = np.full((128, 128), RED, dtype=ml_dtypes.bfloat16)
    rotP = _rot_perm()
    bf = ml_dtypes.bfloat16
    f8 = ml_dtypes.float8_e4m3

    def planes(a, scale):
        hi = (scale * a).astype(f8)
        lo = (scale * a - hi.astype(np.float32)).astype(f8)
        return hi, lo

    hs = np.asarray(hidden_states, dtype=np.float32)
    wq = np.asarray(wq, dtype=np.float32)
    wk = np.asarray(wk, dtype=np.float32)
    wv = np.asarray(wv, dtype=np.float32)
    wo = np.asarray(wo, dtype=np.float32)
    xplanes = [planes(np.ascontiguousarray(hs[b].T), AX) for b in range(B)]

    in_maps = []
    for core in range(NCORES):
        b, g = divmod(core, G)
        wqh_, wql_ = planes(np.ascontiguousarray(wq[:, GD * g:GD * (g + 1)]), AW)
        wkh_, wkl_ = planes(np.ascontiguousarray(wk[:, D * g:D * (g + 1)]), AW)
        wvh_, wvl_ = planes(np.ascontiguousarray(wv[:, D * g:D * (g + 1)]), AW)
        woh_, wol_ = planes(np.ascontiguousarray(wo[GD * g:GD * (g + 1), :]), AW)
        in_maps.append({
            "xh": xplanes[b][0].reshape(NE, 128, S),
            "xl": xplanes[b][1].reshape(NE, 128, S),
            "wqh": wqh_.reshape(NE, 128, GD),
            "wql": wql_.reshape(NE, 128, GD),
            "wkh": wkh_.reshape(NE, 128, D),
            "wkl": wkl_.reshape(NE, 128, D),
            "wvh": wvh_.reshape(NE, 128, D),
            "wvl": wvl_.reshape(NE, 128, D),
            "woh": woh_.reshape(G, 128, E),
            "wol": wol_.reshape(G, 128, E),
            "cosT": cosT,
            "sinT": sinT,
            "rotP": rotP,
            "ident": ident,
            "onesb": onesb,
        })

    res = run_bass_kernel_spmd(_NC, in_maps, list(range(NCORES)))
    LAST_RESULTS = res
    outs = [np.asarray(res.results[i]["out"], dtype=np.float32)
            for i in range(NCORES)]
    full = np.stack([sum(outs[b * G:(b + 1) * G]) for b in range(B)], axis=0)
    return (full / PSC).astype(np.float32)



# revision 6
# speedup vs baseline: 1.1220x; 1.1220x over previous
"""GQA (16 q-heads / 4 kv-heads, D=128, S=2048, E=2048, B=2) on 8 trn2 cores.

Sharding: core = 4*b + g  (b in {0,1} batch, g in {0..3} kv-head group).
Each core computes its batch's 4 query heads (one kv group) end-to-end and
the host sums the 4 partial o_proj outputs per batch.

v4 (balanced Act/PE, paired exp, direct stores):
  - Host pre-arranges all tensors partition-major so every DMA moves >=512B
    contiguous runs (no strided-transpose DMAs); wq/wk/wv/wo resident in
    SBUF fp8 hi/lo planes, x resident per chunk.
  - V projected directly into natural [keys, D] layout (x-tile stationary,
    wv moving) -- no PE transpose, no staging copy.
  - Attention in four 512-wide query chunks; key tiles processed in PAIRS:
    scores for tiles 2j,2j+1 -> one [128,2,512] PSUM tile -> ONE Act exp
    instruction (halves Act instruction overhead); AV per tile in bf16.
    Pipeline depth 2 pairs: scp(j) + drained fill units run before
    avp(j-2), so exp latency is fully hidden.
  - Deferred projection/o_proj work queued as ~2-matmul units and drained a
    few per pair-iteration: B0<-Q1, B1<-Q2+O(c0), B2<-Q3+O(c1), B3<-O(c2),
    tail<-O(c3).  o_proj accumulates in PSUM and stores PSUM->DRAM direct.
"""

import numpy as np
import ml_dtypes

import concourse.bass as bass
import concourse.bacc as bacc
import concourse.mybir as mybir
import concourse.tile as tile
from concourse.bass_utils import run_bass_kernel_spmd

B, S, E = 2, 2048, 2048
H, HKV, D = 16, 4, 128
G = H // HKV          # 4 query heads per kv group
GD = G * D            # 512 channels per group
NCORES = 8
SCALE = 1.0 / float(np.sqrt(D))
ROPE_BASE = 10000.0
AX = 16.0             # fp8 plane scale for x
AW = 64.0             # fp8 plane scale for wq/wk/wv/wo
PSC = AX * AW         # q/k/v come out scaled by PSC
SCALE_EFF = SCALE / (PSC * PSC)   # folds the q*k scale into exp
AO = 16.0             # fp8 plane scale for the normalized attention output
# the softmax reduce uses (PSC/AO)-valued "ones", so ot = AO * attn_out and
# the o_proj result comes out scaled by AO*AW = PSC; the host divides once.
RED = PSC / AO

NE = E // 128         # 16 e-blocks (contraction for projections)
NC4 = S // 512        # 4 position chunks of 512
NST = S // 128        # 16 sk-tiles of 128
NP = NST // 2         # 8 sk-tile PAIRS

F32 = mybir.dt.float32
BF16 = mybir.dt.bfloat16
FP8 = mybir.dt.float8e4
DR = mybir.MatmulPerfMode.DoubleRow
AF = mybir.ActivationFunctionType
OP = mybir.AluOpType

PLANES = ((0, 0), (0, 1), (1, 0))   # (w_plane, x_plane): HH, HL, LH


def _emit(nc, tc, xh, xl, wqh, wql, wkh, wkl, wvh, wvl, woh, wol, cosT,
          sinT, rotP, onesb, out):
    from contextlib import ExitStack
    import collections
    es = ExitStack()
    with es:
        cpool = es.enter_context(tc.tile_pool(name="const", bufs=1))
        xpool = es.enter_context(tc.tile_pool(name="xs", bufs=1))
        rpool = es.enter_context(tc.tile_pool(name="rope", bufs=2))
        etpool = es.enter_context(tc.tile_pool(name="et", bufs=4))
        bcspool = es.enter_context(tc.tile_pool(name="bcs", bufs=2))
        dnpool = es.enter_context(tc.tile_pool(name="dn", bufs=2))
        rcpool = es.enter_context(tc.tile_pool(name="rc", bufs=2))
        otpool = es.enter_context(tc.tile_pool(name="ot", bufs=2))
        ostgpool = es.enter_context(tc.tile_pool(name="ostg", bufs=3))
        pssc = es.enter_context(
            tc.tile_pool(name="pssc", bufs=2, space=bass.MemorySpace.PSUM))
        psav = es.enter_context(
            tc.tile_pool(name="psav", bufs=1, space=bass.MemorySpace.PSUM))
        psmx = es.enter_context(
            tc.tile_pool(name="psmx", bufs=3, space=bass.MemorySpace.PSUM))

        # ---- persistent SBUF tensors ----
        rp_sb = cpool.tile([128, 128], BF16, tag="rp")
        ones_sb = cpool.tile([128, 1], BF16, tag="ones")
        cos_sb = cpool.tile([D, S], BF16, tag="cos")
        sin_sb = cpool.tile([D, S], BF16, tag="sin")
        wk_t = [cpool.tile([128, NE, D], FP8, tag=f"wkt{i}", name=f"wkt{i}")
                for i in range(2)]
        wv_t = [cpool.tile([128, NE, D], FP8, tag=f"wvt{i}", name=f"wvt{i}")
                for i in range(2)]
        wq_t = [cpool.tile([128, NE, GD], FP8, tag=f"wqt{i}", name=f"wqt{i}")
                for i in range(2)]
        wo_t = [cpool.tile([128, G, E], FP8, tag=f"wot{i}", name=f"wot{i}")
                for i in range(2)]
        kt = cpool.tile([D, S], BF16, tag="kt")
        qt = [cpool.tile([D, S], BF16, tag=f"qt{h}", name=f"qt{h}")
              for h in range(G)]
        vn = cpool.tile([128, NST, D], BF16, tag="vn")

        xt = {}

        def load_x(c4, nsplit):
            for i, t in enumerate((xh, xl)):
                xtile = xpool.tile([128, NE, 512], FP8, tag=f"x{c4}_{i}",
                                   name=f"x{c4}_{i}")
                step = NE // nsplit
                for s in range(nsplit):
                    nc.sync.dma_start(
                        out=xtile[:, s * step:(s + 1) * step, :],
                        in_=t.ap()[:, s * step:(s + 1) * step,
                                   c4 * 512:(c4 + 1) * 512])
                xt[(c4, i)] = xtile

        # ---- DMA schedule (dependency order; all contiguous-run >=512B) ----
        for i, t in enumerate((wkh, wkl)):
            nc.sync.dma_start(out=wk_t[i][:], in_=t.ap())
        load_x(0, 4)
        nc.sync.dma_start(out=rp_sb[:], in_=rotP.ap())
        nc.sync.dma_start(out=cos_sb[:], in_=cosT.ap())
        nc.sync.dma_start(out=sin_sb[:], in_=sinT.ap())
        for i, t in enumerate((wvh, wvl)):
            nc.sync.dma_start(out=wv_t[i][:], in_=t.ap())
        load_x(1, 2)
        for i, t in enumerate((wqh, wql)):
            for s in range(2):
                nc.sync.dma_start(
                    out=wq_t[i][:, s * 8:(s + 1) * 8, :],
                    in_=t.ap()[:, s * 8:(s + 1) * 8, :])
        nc.sync.dma_start(out=ones_sb[:], in_=onesb.ap())
        load_x(3, 2)
        load_x(2, 2)
        for i, t in enumerate((woh, wol)):
            for s in range(2):
                nc.sync.dma_start(
                    out=wo_t[i][:, s * 2:(s + 1) * 2, :],
                    in_=t.ap()[:, s * 2:(s + 1) * 2, :])

        # ---- fill-unit queues ----
        fill = collections.deque()
        tailq = collections.deque()

        def drain(n, q=None):
            q = fill if q is None else q
            while n > 0 and q:
                q.popleft()()
                n -= 1

        # ---- rope: rotate_half as signed-permutation matmul ----
        def rope_start(ps, eng):
            qraw = rpool.tile([128, 512], BF16, tag="qraw")
            if eng == 'act':
                nc.scalar.copy(qraw[:], ps[:])
            else:
                nc.vector.tensor_copy(qraw[:], ps[:])
            return qraw

        def rope_finish(dst, qraw, sl):
            tmc = rpool.tile([128, 512], BF16, tag="tmc")
            t2 = rpool.tile([128, 512], BF16, tag="t2")
            rot = psmx.tile([128, 512], F32, tag="mx", name="rot")
            nc.tensor.matmul(rot[:], rp_sb[:], qraw[:], start=True, stop=True)
            nc.gpsimd.tensor_tensor(tmc[:], qraw[:], cos_sb[:, sl], OP.mult)
            nc.vector.tensor_tensor(t2[:], rot[:], sin_sb[:, sl], OP.mult)
            nc.vector.tensor_tensor(dst, tmc[:], t2[:], OP.add)

        # ---- projections (fp8 DoubleRow, 3 quant planes) ----
        def proj_mms(wt, cslice, c4):
            mms = []
            for wi, xi in PLANES:
                for p in range(NE // 2):
                    mms.append((wt[wi][:, 2 * p:2 * p + 2, cslice],
                                xt[(c4, xi)][:, 2 * p:2 * p + 2, :]))
            return mms

        def kproj(c4):
            sl = slice(c4 * 512, (c4 + 1) * 512)
            ps = psmx.tile([128, 512], F32, tag="mx", name="ps")
            mms = proj_mms(wk_t, slice(0, D), c4)
            for i, (wa, xa) in enumerate(mms):
                nc.tensor.matmul(ps[:], wa, xa, perf_mode=DR,
                                 start=(i == 0), stop=(i == len(mms) - 1))
            return ps, sl

        def vproj_mms(c4):
            vp = psmx.tile([128, 4, 128], F32, tag="mx", name="vp")
            for i in range(4):
                ksl = slice(i * 128, (i + 1) * 128)
                j = 0
                for wi, xi in PLANES:
                    for p in range(NE // 2):
                        nc.tensor.matmul(
                            vp[:, i, :],
                            xt[(c4, xi)][:, 2 * p:2 * p + 2, ksl],
                            wv_t[wi][:, 2 * p:2 * p + 2, :],
                            perf_mode=DR, start=(j == 0), stop=(j == 23))
                        j += 1
            return vp

        def vn_copy(c4, vp, eng):
            dst = vn[:, c4 * 4:(c4 + 1) * 4, :]
            if eng == 'act':
                nc.scalar.copy(dst, vp[:])
            elif eng == 'pool':
                nc.gpsimd.tensor_copy(dst, vp[:])
            else:
                nc.vector.tensor_copy(dst, vp[:])

        def qproj(h, c4):
            # phase-A inline Q projection; rope finished by caller interleave
            sl = slice(c4 * 512, (c4 + 1) * 512)
            ps = psmx.tile([128, 512], F32, tag="mx", name="ps")
            mms = proj_mms(wq_t, slice(h * D, (h + 1) * D), c4)
            for i, (wa, xa) in enumerate(mms):
                nc.tensor.matmul(ps[:], wa, xa, perf_mode=DR,
                                 start=(i == 0), stop=(i == len(mms) - 1))
            return ps, sl

        def enqueue_qproj(h, c4):
            sl = slice(c4 * 512, (c4 + 1) * 512)
            state = {}
            nmm = 24

            def mk(j):
                def unit():
                    if j == 0:
                        state['ps'] = psmx.tile([128, 512], F32, tag="mx",
                                                name="ps")
                        state['mms'] = proj_mms(
                            wq_t, slice(h * D, (h + 1) * D), c4)
                    ps = state['ps']
                    for jj in (2 * j, 2 * j + 1):
                        wa, xa = state['mms'][jj]
                        nc.tensor.matmul(ps[:], wa, xa, perf_mode=DR,
                                         start=(jj == 0), stop=(jj == nmm - 1))
                return unit
            for j in range(nmm // 2):
                fill.append(mk(j))

            def fin():
                qraw = rope_start(state['ps'], 'dve')
                rope_finish(qt[h][:, sl], qraw, sl)
            fill.append(fin)

        # ---- o_proj: ot (fp8 hi/lo planes) @ wo, direct PSUM->DRAM ----
        def enqueue_oproj(ci, oth, otl, q):
            off = ci * 512
            for st in range(4):
                ssl = slice(st * 128, (st + 1) * 128)
                for eo in range(4):
                    esl = slice(eo * 512, (eo + 1) * 512)
                    state = {}
                    mms = []
                    for wi_src, wi in ((oth, 0), (oth, 1), (otl, 0)):
                        for i in range(2):
                            mms.append((wi_src[:, 2 * i:2 * i + 2, ssl],
                                        wo_t[wi][:, 2 * i:2 * i + 2, esl]))

                    def mk(st, eo, j, mms=mms, state=state):
                        def unit():
                            if j == 0:
                                state['op'] = psmx.tile([128, 512], F32,
                                                        tag="mx", name="op")
                            op = state['op']
                            for jj in (2 * j, 2 * j + 1):
                                oa, wa = mms[jj]
                                nc.tensor.matmul(op[:], oa, wa, perf_mode=DR,
                                                 start=(jj == 0),
                                                 stop=(jj == 5))
                            if j == 2:
                                ostg = ostgpool.tile([128, 512], BF16,
                                                     tag="ostg")
                                eng = (st + eo) % 3
                                if eng == 0:
                                    nc.scalar.copy(ostg[:], op[:])
                                elif eng == 1:
                                    nc.vector.tensor_copy(ostg[:], op[:])
                                else:
                                    nc.gpsimd.tensor_copy(ostg[:], op[:])
                                nc.sync.dma_start(
                                    out=out.ap()[off + st * 128:
                                                 off + (st + 1) * 128,
                                                 eo * 512:(eo + 1) * 512],
                                    in_=ostg[:])
                        return unit
                    for j in range(3):
                        q.append(mk(st, eo, j))

        # ---- K/V as fill units (not used in final schedule; kept simple) --

        # ---- attention: paired key tiles, depth-2 pair pipeline ----
        def attn_head(ci, h, oth, otl, drain_n):
            off = ci * 512
            qsl = slice(off, off + 512)
            dn = dnpool.tile([128, 512], BF16, tag="dn")
            av = psav.tile([D, 512], F32, tag="av")

            def scp_exp(j):
                sc = pssc.tile([128, 2, 512], F32, tag="sc")
                for tt in range(2):
                    t = 2 * j + tt
                    nc.tensor.matmul(sc[:, tt, :],
                                     kt[:, t * 128:(t + 1) * 128],
                                     qt[h][:, qsl], start=True, stop=True)
                et = etpool.tile([128, 2, 512], BF16, tag="et")
                nc.scalar.activation(et[:], sc[:], AF.Exp, scale=SCALE_EFF)
                return et

            def avp(j, et):
                for tt in range(2):
                    t = 2 * j + tt
                    nc.tensor.matmul(av[:], vn[:, t, :], et[:, tt, :],
                                     start=(t == 0), stop=(t == NST - 1))

            def dnp(j, et):
                for tt in range(2):
                    if j == 0 and tt == 0:
                        nc.vector.tensor_copy(dn[:], et[:, 0, :])
                    else:
                        nc.vector.tensor_tensor(dn[:], dn[:], et[:, tt, :],
                                                OP.add)

            ets = {}
            ets[0] = scp_exp(0)
            ets[1] = scp_exp(1)
            drain(drain_n)
            for j in range(2, NP):
                ets[j] = scp_exp(j)
                drain(drain_n)
                avp(j - 2, ets[j - 2])
                dnp(j - 2, ets[j - 2])
                del ets[j - 2]
            drain(drain_n)
            avp(NP - 2, ets[NP - 2])
            dnp(NP - 2, ets[NP - 2])
            drain(drain_n)
            avp(NP - 1, ets[NP - 1])
            dnp(NP - 1, ets[NP - 1])
            drain(4)
            # epilogue: denominator reduce, reciprocal, normalize, fp8 planes
            sm = psmx.tile([1, 512], F32, tag="mx", name="sm")
            nc.tensor.matmul(sm[:], ones_sb[:, 0:1], dn[:],
                             start=True, stop=True)
            rc = rcpool.tile([1, 512], BF16, tag="rc")
            with nc.allow_low_precision(reason="bf16 softmax denom recip"):
                nc.vector.reciprocal(rc[:], sm[:])
            bcs = bcspool.tile([128, 512], BF16, tag="bcs")
            nc.gpsimd.partition_broadcast(bcs[:], rc[:])
            otf = rpool.tile([D, 512], F32, tag="otf")
            nc.vector.tensor_tensor(otf[:], av[:], bcs[:], OP.mult)
            nc.scalar.copy(oth[:, h, :], otf[:])
            nc.vector.tensor_tensor(otl[:, h, :], otf[:], oth[:, h, :],
                                    OP.subtract)

        # ================= phase A: K all, V all, Q chunk 0 =================
        ps, sl = kproj(0)
        qraw = rope_start(ps, 'act')
        vp = vproj_mms(0)
        rope_finish(kt[:, sl], qraw, sl)
        vn_copy(0, vp, 'act')
        for c4 in (1, 2, 3):
            ps, sl = kproj(c4)
            qraw = rope_start(ps, 'act')
            vp = vproj_mms(c4)
            rope_finish(kt[:, sl], qraw, sl)
            vn_copy(c4, vp, 'act')
        prev = None
        for h in range(G):
            ps, sl = qproj(h, 0)
            if prev is not None:
                ph, pq, psl = prev
                rope_finish(qt[ph][:, psl], pq, psl)
            qraw = rope_start(ps, 'act')
            prev = (h, qraw, sl)
        ph, pq, psl = prev
        rope_finish(qt[ph][:, psl], pq, psl)

        # ================= B windows: attention + drained fills =============
        def ot_planes(ci):
            hi = otpool.tile([128, G, 512], FP8, tag="oth", name=f"oth{ci}")
            lo = otpool.tile([128, G, 512], FP8, tag="otl", name=f"otl{ci}")
            return hi, lo

        planes = {}
        # B0: fills = Q chunk 1
        planes[0] = ot_planes(0)
        for hq in range(G):
            enqueue_qproj(hq, 1)
        for h in range(G):
            attn_head(0, h, planes[0][0], planes[0][1], 1)
        drain(len(fill))
        # B1: fills = Q chunk 2 + o_proj of chunk 0
        planes[1] = ot_planes(1)
        for h in range(G):
            if h == 0:
                for hq in range(G):
                    enqueue_qproj(hq, 2)
            elif h == 1:
                enqueue_oproj(0, planes[0][0], planes[0][1], fill)
            attn_head(1, h, planes[1][0], planes[1][1], 3)
        drain(len(fill))
        # B2: fills = Q chunk 3 + o_proj of chunk 1
        planes[2] = ot_planes(2)
        for h in range(G):
            if h == 0:
                for hq in range(G):
                    enqueue_qproj(hq, 3)
            elif h == 1:
                enqueue_oproj(1, planes[1][0], planes[1][1], fill)
            attn_head(2, h, planes[2][0], planes[2][1], 3)
        drain(len(fill))
        # B3: fills = o_proj of chunk 2
        planes[3] = ot_planes(3)
        for h in range(G):
            if h == 0:
                enqueue_oproj(2, planes[2][0], planes[2][1], fill)
            attn_head(3, h, planes[3][0], planes[3][1], 2)
        drain(len(fill))
        # tail: o_proj of chunk 3
        enqueue_oproj(3, planes[3][0], planes[3][1], tailq)
        drain(len(tailq), tailq)


def _build():
    nc = bacc.Bacc("TRN2", target_bir_lowering=False, debug=False,
                   num_devices=NCORES)
    xh = nc.dram_tensor("xh", [128, NE, S], FP8, kind="ExternalInput")
    xl = nc.dram_tensor("xl", [128, NE, S], FP8, kind="ExternalInput")
    wqh = nc.dram_tensor("wqh", [128, NE, GD], FP8, kind="ExternalInput")
    wql = nc.dram_tensor("wql", [128, NE, GD], FP8, kind="ExternalInput")
    wkh = nc.dram_tensor("wkh", [128, NE, D], FP8, kind="ExternalInput")
    wkl = nc.dram_tensor("wkl", [128, NE, D], FP8, kind="ExternalInput")
    wvh = nc.dram_tensor("wvh", [128, NE, D], FP8, kind="ExternalInput")
    wvl = nc.dram_tensor("wvl", [128, NE, D], FP8, kind="ExternalInput")
    woh = nc.dram_tensor("woh", [128, G, E], FP8, kind="ExternalInput")
    wol = nc.dram_tensor("wol", [128, G, E], FP8, kind="ExternalInput")
    cosT = nc.dram_tensor("cosT", [D, S], BF16, kind="ExternalInput")
    sinT = nc.dram_tensor("sinT", [D, S], BF16, kind="ExternalInput")
    rotP = nc.dram_tensor("rotP", [128, 128], BF16, kind="ExternalInput")
    onesb = nc.dram_tensor("onesb", [128, 1], BF16, kind="ExternalInput")
    out = nc.dram_tensor("out", [S, E], BF16, kind="ExternalOutput")
    with tile.TileContext(nc) as tc:
        _emit(nc, tc, xh, xl, wqh, wql, wkh, wkl, wvh, wvl, woh, wol, cosT,
              sinT, rotP, onesb, out)
    nc.compile()
    return nc


def _rope_tables():
    inv = 1.0 / (ROPE_BASE ** (np.arange(0, D, 2, dtype=np.float64) / D))
    t = np.arange(S, dtype=np.float64)
    freqs = t[:, None] * inv[None, :]                    # [S, D/2]
    emb = np.concatenate([freqs, freqs], axis=-1)        # [S, D]
    cosT = np.cos(emb).T.astype(ml_dtypes.bfloat16)      # [D, S]
    sinT = np.sin(emb).T.astype(ml_dtypes.bfloat16)
    return np.ascontiguousarray(cosT), np.ascontiguousarray(sinT)


def _rot_perm():
    # rot(q)[d] = -q[d+64] for d<64, +q[d-64] for d>=64, as a stationary
    # matmul operand: rot = P^T @ q with P[k, m] below.
    p = np.zeros((128, 128), dtype=ml_dtypes.bfloat16)
    for d in range(64):
        p[d + 64, d] = -1.0
        p[d, d + 64] = 1.0
    return p


def _pm(a, nblk):
    """[K, M] -> partition-major [128, nblk, M] (K = nblk*128)."""
    k, m = a.shape
    return np.ascontiguousarray(a.reshape(nblk, 128, m).transpose(1, 0, 2))


_NC = None
LAST_RESULTS = None


def kernel(hidden_states, wq, wk, wv, wo):
    global _NC, LAST_RESULTS
    if _NC is None:
        _NC = _build()
    cosT, sinT = _rope_tables()
    onesb = np.full((128, 1), RED, dtype=ml_dtypes.bfloat16)
    rotP = _rot_perm()
    f8 = ml_dtypes.float8_e4m3

    def planes(a, scale):
        hi = (scale * a).astype(f8)
        lo = (scale * a - hi.astype(np.float32)).astype(f8)
        return hi, lo

    hs = np.asarray(hidden_states, dtype=np.float32)
    wq = np.asarray(wq, dtype=np.float32)
    wk = np.asarray(wk, dtype=np.float32)
    wv = np.asarray(wv, dtype=np.float32)
    wo = np.asarray(wo, dtype=np.float32)
    xplanes = []
    for b in range(B):
        hi, lo = planes(np.ascontiguousarray(hs[b].T), AX)
        xplanes.append((_pm(hi, NE), _pm(lo, NE)))

    in_maps = []
    for core in range(NCORES):
        b, g = divmod(core, G)
        wqh_, wql_ = planes(wq[:, GD * g:GD * (g + 1)], AW)
        wkh_, wkl_ = planes(wk[:, D * g:D * (g + 1)], AW)
        wvh_, wvl_ = planes(wv[:, D * g:D * (g + 1)], AW)
        woh_, wol_ = planes(wo[GD * g:GD * (g + 1), :], AW)
        in_maps.append({
            "xh": xplanes[b][0],
            "xl": xplanes[b][1],
            "wqh": _pm(wqh_, NE),
            "wql": _pm(wql_, NE),
            "wkh": _pm(wkh_, NE),
            "wkl": _pm(wkl_, NE),
            "wvh": _pm(wvh_, NE),
            "wvl": _pm(wvl_, NE),
            "woh": _pm(woh_, G),
            "wol": _pm(wol_, G),
            "cosT": cosT,
            "sinT": sinT,
            "rotP": rotP,
            "onesb": onesb,
        })

    res = run_bass_kernel_spmd(_NC, in_maps, list(range(NCORES)))
    LAST_RESULTS = res
    outs = [np.asarray(res.results[i]["out"], dtype=np.float32)
            for i in range(NCORES)]
    full = np.stack([sum(outs[b * G:(b + 1) * G]) for b in range(B)], axis=0)
    return (full / PSC).astype(np.float32)


# revision 10
# speedup vs baseline: 1.2082x; 1.0769x over previous
"""GQA (16 q-heads / 4 kv-heads, D=128, S=2048, E=2048, B=2) on 8 trn2 cores.

Sharding: core = 4*b + g  (b in {0,1} batch, g in {0..3} kv-head group).
Each core computes its batch's 4 query heads (one kv group) end-to-end and
the host sums the 4 partial o_proj outputs per batch.

v4 (balanced Act/PE, paired exp, direct stores):
  - Host pre-arranges all tensors partition-major so every DMA moves >=512B
    contiguous runs (no strided-transpose DMAs); wq/wk/wv/wo resident in
    SBUF fp8 hi/lo planes, x resident per chunk.
  - V projected directly into natural [keys, D] layout (x-tile stationary,
    wv moving) -- no PE transpose, no staging copy.
  - Attention in four 512-wide query chunks; key tiles processed in PAIRS:
    scores for tiles 2j,2j+1 -> one [128,2,512] PSUM tile -> ONE Act exp
    instruction (halves Act instruction overhead); AV per tile in bf16.
    Pipeline depth 2 pairs: scp(j) + drained fill units run before
    avp(j-2), so exp latency is fully hidden.
  - Deferred projection/o_proj work queued as ~2-matmul units and drained a
    few per pair-iteration: B0<-Q1, B1<-Q2+O(c0), B2<-Q3+O(c1), B3<-O(c2),
    tail<-O(c3).  o_proj accumulates in PSUM and stores PSUM->DRAM direct.
"""

import numpy as np
import ml_dtypes

import concourse.bass as bass
import concourse.bacc as bacc
import concourse.mybir as mybir
import concourse.tile as tile
from concourse.bass_utils import run_bass_kernel_spmd

B, S, E = 2, 2048, 2048
H, HKV, D = 16, 4, 128
G = H // HKV          # 4 query heads per kv group
GD = G * D            # 512 channels per group
NCORES = 8
SCALE = 1.0 / float(np.sqrt(D))
ROPE_BASE = 10000.0
AX = 16.0             # fp8 plane scale for x
AW = 64.0             # fp8 plane scale for wq/wk/wv/wo
PSC = AX * AW         # q/k/v come out scaled by PSC
SCALE_EFF = SCALE / (PSC * PSC)   # folds the q*k scale into exp
AO = 16.0             # fp8 plane scale for the normalized attention output
# the softmax reduce uses (PSC/AO)-valued "ones", so ot = AO * attn_out and
# the o_proj result comes out scaled by AO*AW = PSC; the host divides once.
RED = PSC / AO

NE = E // 128         # 16 e-blocks (contraction for projections)
NC4 = S // 512        # 4 position chunks of 512
NST = S // 128        # 16 sk-tiles of 128
NP = NST // 2         # 8 sk-tile PAIRS

F32 = mybir.dt.float32
BF16 = mybir.dt.bfloat16
FP8 = mybir.dt.float8e4
DR = mybir.MatmulPerfMode.DoubleRow
AF = mybir.ActivationFunctionType
OP = mybir.AluOpType

PLANES = ((0, 0), (0, 1), (1, 0))   # (w_plane, x_plane): HH, HL, LH


def _emit(nc, tc, xh, xl, wqh, wql, wkh, wkl, wvh, wvl, woh, wol, cosT,
          sinT, rotP, onesb, out):
    from contextlib import ExitStack
    import collections
    es = ExitStack()
    with es:
        cpool = es.enter_context(tc.tile_pool(name="const", bufs=1))
        xpool = es.enter_context(tc.tile_pool(name="xs", bufs=1))
        rpool = es.enter_context(tc.tile_pool(name="rope", bufs=2))
        etpool = es.enter_context(tc.tile_pool(name="et", bufs=4))
        bcspool = es.enter_context(tc.tile_pool(name="bcs", bufs=2))
        dnpool = es.enter_context(tc.tile_pool(name="dn", bufs=2))
        rcpool = es.enter_context(tc.tile_pool(name="rc", bufs=2))
        otpool = es.enter_context(tc.tile_pool(name="ot", bufs=2))
        ostgpool = es.enter_context(tc.tile_pool(name="ostg", bufs=6))
        pssc = es.enter_context(
            tc.tile_pool(name="pssc", bufs=2, space=bass.MemorySpace.PSUM))
        psav = es.enter_context(
            tc.tile_pool(name="psav", bufs=1, space=bass.MemorySpace.PSUM))
        psmx = es.enter_context(
            tc.tile_pool(name="psmx", bufs=3, space=bass.MemorySpace.PSUM))

        # ---- persistent SBUF tensors ----
        rp_sb = cpool.tile([128, 128], BF16, tag="rp")
        ones_sb = cpool.tile([128, 1], BF16, tag="ones")
        cos_sb = cpool.tile([D, S], BF16, tag="cos")
        sin_sb = cpool.tile([D, S], BF16, tag="sin")
        wk_t = [cpool.tile([128, NE, D], FP8, tag=f"wkt{i}", name=f"wkt{i}")
                for i in range(2)]
        wv_t = [cpool.tile([128, NE, D], FP8, tag=f"wvt{i}", name=f"wvt{i}")
                for i in range(2)]
        wq_t = [cpool.tile([128, NE, GD], FP8, tag=f"wqt{i}", name=f"wqt{i}")
                for i in range(2)]
        wo_t = [cpool.tile([128, G, E], FP8, tag=f"wot{i}", name=f"wot{i}")
                for i in range(2)]
        kt = cpool.tile([D, S], BF16, tag="kt")
        qt = [cpool.tile([D, S], BF16, tag=f"qt{h}", name=f"qt{h}")
              for h in range(G)]
        vn = cpool.tile([128, NST, D], BF16, tag="vn")

        xt = {}

        def load_x(c4, nsplit):
            for i, t in enumerate((xh, xl)):
                xtile = xpool.tile([128, NE, 512], FP8, tag=f"x{c4}_{i}",
                                   name=f"x{c4}_{i}")
                step = NE // nsplit
                for s in range(nsplit):
                    nc.sync.dma_start(
                        out=xtile[:, s * step:(s + 1) * step, :],
                        in_=t.ap()[:, s * step:(s + 1) * step,
                                   c4 * 512:(c4 + 1) * 512])
                xt[(c4, i)] = xtile

        # ---- DMA schedule (dependency order; contiguous runs >=512B) ----
        for i, t in enumerate((wkh, wkl)):
            nc.sync.dma_start(out=wk_t[i][:], in_=t.ap())
        load_x(0, 4)
        nc.sync.dma_start(out=rp_sb[:], in_=rotP.ap())
        for i, t in enumerate((wvh, wvl)):
            nc.sync.dma_start(out=wv_t[i][:], in_=t.ap())

        def load_cs(c4):
            sl = slice(c4 * 512, (c4 + 1) * 512)
            nc.sync.dma_start(out=cos_sb[:, sl], in_=cosT.ap()[:, sl])
            nc.sync.dma_start(out=sin_sb[:, sl], in_=sinT.ap()[:, sl])

        load_cs(0)
        for i, t in enumerate((wqh, wql)):
            for sp in range(2):
                nc.sync.dma_start(
                    out=wq_t[i][:, sp * 8:(sp + 1) * 8, :],
                    in_=t.ap()[:, sp * 8:(sp + 1) * 8, :])
        load_x(1, 2)
        load_cs(1)
        load_x(2, 2)
        load_cs(2)
        load_x(3, 2)
        load_cs(3)
        nc.sync.dma_start(out=ones_sb[:], in_=onesb.ap())
        for i, t in enumerate((woh, wol)):
            for sp in range(2):
                nc.sync.dma_start(
                    out=wo_t[i][:, sp * 2:(sp + 1) * 2, :],
                    in_=t.ap()[:, sp * 2:(sp + 1) * 2, :])

        # ---- fill-unit queues (labelled; require() force-drains FIFO
        # until a label's units are all emitted -- keeps emission order
        # consistent with data dependencies) ----
        fill = collections.deque()
        tailq = collections.deque()
        pending = collections.Counter()

        def enq(label, fn, q=None):
            (fill if q is None else q).append((label, fn))
            pending[label] += 1

        def drain(n, q=None):
            q = fill if q is None else q
            while n > 0 and q:
                lab, fn = q.popleft()
                fn()
                pending[lab] -= 1
                n -= 1

        def require(label):
            while pending.get(label, 0) > 0:
                lab, fn = fill.popleft()
                fn()
                pending[lab] -= 1

        # ---- rope: rotate_half as signed-permutation matmul ----
        def rope_start(ps, eng):
            qraw = rpool.tile([128, 512], BF16, tag="qraw")
            if eng == 'act':
                nc.scalar.copy(qraw[:], ps[:])
            else:
                nc.vector.tensor_copy(qraw[:], ps[:])
            return qraw

        def rope_finish(dst, qraw, sl):
            tmc = rpool.tile([128, 512], BF16, tag="tmc")
            t2 = rpool.tile([128, 512], BF16, tag="t2")
            rot = psmx.tile([128, 512], F32, tag="mx", name="rot")
            nc.tensor.matmul(rot[:], rp_sb[:], qraw[:], start=True, stop=True)
            nc.gpsimd.tensor_tensor(tmc[:], qraw[:], cos_sb[:, sl], OP.mult)
            nc.vector.tensor_tensor(t2[:], rot[:], sin_sb[:, sl], OP.mult)
            nc.vector.tensor_tensor(dst, tmc[:], t2[:], OP.add)

        # ---- projections (fp8 DoubleRow, 3 quant planes) ----
        def proj_mms(wt, cslice, c4):
            mms = []
            for wi, xi in PLANES:
                for p in range(NE // 2):
                    mms.append((wt[wi][:, 2 * p:2 * p + 2, cslice],
                                xt[(c4, xi)][:, 2 * p:2 * p + 2, :]))
            return mms

        def kproj(c4):
            sl = slice(c4 * 512, (c4 + 1) * 512)
            ps = psmx.tile([128, 512], F32, tag="mx", name="ps")
            mms = proj_mms(wk_t, slice(0, D), c4)
            for i, (wa, xa) in enumerate(mms):
                nc.tensor.matmul(ps[:], wa, xa, perf_mode=DR,
                                 start=(i == 0), stop=(i == len(mms) - 1))
            return ps, sl

        def vproj_mms(c4):
            vp = psmx.tile([128, 4, 128], F32, tag="mx", name="vp")
            for i in range(4):
                ksl = slice(i * 128, (i + 1) * 128)
                j = 0
                for wi, xi in PLANES:
                    for p in range(NE // 2):
                        nc.tensor.matmul(
                            vp[:, i, :],
                            xt[(c4, xi)][:, 2 * p:2 * p + 2, ksl],
                            wv_t[wi][:, 2 * p:2 * p + 2, :],
                            perf_mode=DR, start=(j == 0), stop=(j == 23))
                        j += 1
            return vp

        def vn_copy(c4, vp, eng):
            dst = vn[:, c4 * 4:(c4 + 1) * 4, :]
            if eng == 'act':
                nc.scalar.copy(dst, vp[:])
            elif eng == 'pool':
                nc.gpsimd.tensor_copy(dst, vp[:])
            else:
                nc.vector.tensor_copy(dst, vp[:])

        def qproj(h, c4):
            # phase-A inline Q projection; rope finished by caller interleave
            sl = slice(c4 * 512, (c4 + 1) * 512)
            ps = psmx.tile([128, 512], F32, tag="mx", name="ps")
            mms = proj_mms(wq_t, slice(h * D, (h + 1) * D), c4)
            for i, (wa, xa) in enumerate(mms):
                nc.tensor.matmul(ps[:], wa, xa, perf_mode=DR,
                                 start=(i == 0), stop=(i == len(mms) - 1))
            return ps, sl

        def enqueue_qproj(h, c4):
            sl = slice(c4 * 512, (c4 + 1) * 512)
            lab = f"Q{c4}h{h}"
            state = {}
            nmm = 24

            def mk(j):
                def unit():
                    if j == 0:
                        state['ps'] = psmx.tile([128, 512], F32, tag="mx",
                                                name="ps")
                        state['mms'] = proj_mms(
                            wq_t, slice(h * D, (h + 1) * D), c4)
                    ps = state['ps']
                    for jj in (2 * j, 2 * j + 1):
                        wa, xa = state['mms'][jj]
                        nc.tensor.matmul(ps[:], wa, xa, perf_mode=DR,
                                         start=(jj == 0), stop=(jj == nmm - 1))
                return unit
            for j in range(nmm // 2):
                enq(lab, mk(j))

            def fin():
                qraw = rope_start(state['ps'], 'dve')
                rope_finish(qt[h][:, sl], qraw, sl)
            enq(lab, fin)

        def enqueue_kproj(c4):
            sl = slice(c4 * 512, (c4 + 1) * 512)
            lab = f"K{c4}"
            state = {}
            nmm = 24

            def mk(j):
                def unit():
                    if j == 0:
                        state['ps'] = psmx.tile([128, 512], F32, tag="mx",
                                                name="ps")
                        state['mms'] = proj_mms(wk_t, slice(0, D), c4)
                    ps = state['ps']
                    for jj in (2 * j, 2 * j + 1):
                        wa, xa = state['mms'][jj]
                        nc.tensor.matmul(ps[:], wa, xa, perf_mode=DR,
                                         start=(jj == 0), stop=(jj == nmm - 1))
                return unit
            for j in range(nmm // 2):
                enq(lab, mk(j))

            def fin():
                qraw = rope_start(state['ps'], 'dve')
                rope_finish(kt[:, sl], qraw, sl)
            enq(lab, fin)

        def enqueue_vproj(c4):
            lab = f"V{c4}"
            state = {}

            def mkmm(i, g):
                def unit():
                    if i == 0 and g == 0:
                        state['vp'] = psmx.tile([128, 4, 128], F32, tag="mx",
                                                name="vp")
                    vp = state['vp']
                    ksl = slice(i * 128, (i + 1) * 128)
                    mms = [(xt[(c4, xi)][:, 2 * p:2 * p + 2, ksl],
                            wv_t[wi][:, 2 * p:2 * p + 2, :])
                           for wi, xi in PLANES for p in range(NE // 2)]
                    for jj in range(8 * g, 8 * g + 8):
                        sa, ma = mms[jj]
                        nc.tensor.matmul(vp[:, i, :], sa, ma, perf_mode=DR,
                                         start=(jj == 0), stop=(jj == 23))
                return unit

            def mkcp(i):
                def unit():
                    nc.vector.tensor_copy(vn[:, c4 * 4 + i, :],
                                          state['vp'][:, i, :])
                return unit
            for i in range(4):
                for g in range(3):
                    enq(lab, mkmm(i, g))
                enq(lab, mkcp(i))

        # ---- o_proj: ot (fp8 hi/lo planes) @ wo, PSUM -> bf16 SBUF
        # staging (two 512-col groups share one [128,1024] staging tile and
        # one store) ----
        def enqueue_oproj(ci, oth, otl, q):
            off = ci * 512
            for st in range(4):
                ssl = slice(st * 128, (st + 1) * 128)
                shared = {}
                for eo in range(4):
                    esl = slice(eo * 512, (eo + 1) * 512)
                    state = {}
                    # head-pair i=0 mms first: they only need heads 0/1 of
                    # the ot planes, so the tail can start before the last
                    # head's epilogue lands.
                    mms = []
                    for i in range(2):
                        for src, wi in ((oth, 0), (oth, 1), (otl, 0)):
                            mms.append((src[:, 2 * i:2 * i + 2, ssl],
                                        wo_t[wi][:, 2 * i:2 * i + 2, esl]))

                    def mk(st, eo, j, mms=mms, state=state, shared=shared):
                        def unit():
                            if j == 0:
                                state['op'] = psmx.tile([128, 512], F32,
                                                        tag="mx", name="op")
                            op = state['op']
                            for jj in (2 * j, 2 * j + 1):
                                oa, wa = mms[jj]
                                nc.tensor.matmul(op[:], oa, wa, perf_mode=DR,
                                                 start=(jj == 0),
                                                 stop=(jj == 5))
                            if j == 2:
                                half = eo % 2
                                if half == 0:
                                    shared['ostg'] = ostgpool.tile(
                                        [128, 1024], BF16, tag="ostg",
                                        name="ostg")
                                ostg = shared['ostg']
                                dst = ostg[:, half * 512:(half + 1) * 512]
                                eng = (st + eo // 2) % 3
                                if eng == 0:
                                    nc.scalar.copy(dst, op[:])
                                elif eng == 1:
                                    nc.vector.tensor_copy(dst, op[:])
                                else:
                                    nc.gpsimd.tensor_copy(dst, op[:])
                                if half == 1:
                                    nc.sync.dma_start(
                                        out=out.ap()[off + st * 128:
                                                     off + (st + 1) * 128,
                                                     (eo - 1) * 512:
                                                     (eo + 1) * 512],
                                        in_=ostg[:])
                        return unit
                    for j in range(3):
                        enq(f"O{ci}", mk(st, eo, j), q)

        # ---- K/V as fill units (not used in final schedule; kept simple) --

        # ---- attention: paired key tiles, depth-2 pair pipeline ----
        def attn_head(ci, h, oth, otl, drain_n):
            off = ci * 512
            qsl = slice(off, off + 512)
            require(f"Q{ci}h{h}")
            dn = dnpool.tile([128, 512], BF16, tag="dn")
            av = psav.tile([D, 512], F32, tag="av")

            def scp_exp(j):
                require(f"K{(2 * j + 1) // 4}")
                sc = pssc.tile([128, 2, 512], F32, tag="sc")
                for tt in range(2):
                    t = 2 * j + tt
                    nc.tensor.matmul(sc[:, tt, :],
                                     kt[:, t * 128:(t + 1) * 128],
                                     qt[h][:, qsl], start=True, stop=True)
                et = etpool.tile([128, 2, 512], BF16, tag="et")
                nc.scalar.activation(et[:], sc[:], AF.Exp, scale=SCALE_EFF)
                return et

            def avp(j, et):
                require(f"V{(2 * j + 1) // 4}")
                for tt in range(2):
                    t = 2 * j + tt
                    nc.tensor.matmul(av[:], vn[:, t, :], et[:, tt, :],
                                     start=(t == 0), stop=(t == NST - 1))

            def dnp(j, et):
                for tt in range(2):
                    if j == 0 and tt == 0:
                        nc.vector.tensor_copy(dn[:], et[:, 0, :])
                    else:
                        nc.vector.tensor_tensor(dn[:], dn[:], et[:, tt, :],
                                                OP.add)

            ets = {}
            ets[0] = scp_exp(0)
            ets[1] = scp_exp(1)
            drain(drain_n)
            for j in range(2, NP):
                ets[j] = scp_exp(j)
                drain(drain_n)
                avp(j - 2, ets[j - 2])
                dnp(j - 2, ets[j - 2])
                del ets[j - 2]
            drain(drain_n)
            avp(NP - 2, ets[NP - 2])
            dnp(NP - 2, ets[NP - 2])
            drain(drain_n)
            avp(NP - 1, ets[NP - 1])
            dnp(NP - 1, ets[NP - 1])
            drain(4)
            # epilogue: denominator reduce, reciprocal, normalize, fp8 planes
            sm = psmx.tile([1, 512], F32, tag="mx", name="sm")
            nc.tensor.matmul(sm[:], ones_sb[:, 0:1], dn[:],
                             start=True, stop=True)
            rc = rcpool.tile([1, 512], BF16, tag="rc")
            with nc.allow_low_precision(reason="bf16 softmax denom recip"):
                nc.vector.reciprocal(rc[:], sm[:])
            bcs = bcspool.tile([128, 512], BF16, tag="bcs")
            nc.gpsimd.partition_broadcast(bcs[:], rc[:])
            otf = rpool.tile([D, 512], F32, tag="otf")
            nc.vector.tensor_tensor(otf[:], av[:], bcs[:], OP.mult)
            nc.scalar.copy(oth[:, h, :], otf[:])
            nc.vector.tensor_tensor(otl[:, h, :], otf[:], oth[:, h, :],
                                    OP.subtract)

        # ====== phase A: K/V chunks 0-1, Q chunk 0; rest drains into B0 =====
        ps, sl = kproj(0)
        qraw = rope_start(ps, 'act')
        vp = vproj_mms(0)
        rope_finish(kt[:, sl], qraw, sl)
        vn_copy(0, vp, 'act')
        prev = None
        for h in range(G):
            ps, sl = qproj(h, 0)
            if prev is not None:
                ph, pq, psl = prev
                rope_finish(qt[ph][:, psl], pq, psl)
            qraw = rope_start(ps, 'act')
            prev = (h, qraw, sl)
        ps, sl = kproj(1)
        ph, pq, psl = prev
        rope_finish(qt[ph][:, psl], pq, psl)
        qraw = rope_start(ps, 'act')
        vp = vproj_mms(1)
        rope_finish(kt[:, sl], qraw, sl)
        vn_copy(1, vp, 'act')

        # ================= B windows: attention + drained fills =============
        def ot_planes(ci):
            hi = otpool.tile([128, G, 512], FP8, tag="oth", name=f"oth{ci}")
            lo = otpool.tile([128, G, 512], FP8, tag="otl", name=f"otl{ci}")
            return hi, lo

        planes = {}
        # B0: fills = K/V chunks 2,3 (ledger-paced), then Q chunk 1
        planes[0] = ot_planes(0)
        enqueue_kproj(2)
        enqueue_vproj(2)
        enqueue_kproj(3)
        enqueue_vproj(3)
        for hq in range(G):
            enqueue_qproj(hq, 1)
        for h in range(G):
            attn_head(0, h, planes[0][0], planes[0][1], 6 if h == 0 else 3)
        # B1: fills += Q chunk 2 + o_proj of chunk 0
        planes[1] = ot_planes(1)
        for h in range(G):
            if h == 0:
                for hq in range(G):
                    enqueue_qproj(hq, 2)
            elif h == 1:
                enqueue_oproj(0, planes[0][0], planes[0][1], fill)
            attn_head(1, h, planes[1][0], planes[1][1], 3)
        # B2: fills += Q chunk 3 + o_proj of chunk 1
        planes[2] = ot_planes(2)
        for h in range(G):
            if h == 0:
                for hq in range(G):
                    enqueue_qproj(hq, 3)
            elif h == 1:
                enqueue_oproj(1, planes[1][0], planes[1][1], fill)
            attn_head(2, h, planes[2][0], planes[2][1], 3)
        # B3: fills += o_proj of chunk 2
        planes[3] = ot_planes(3)
        for h in range(G):
            if h == 0:
                enqueue_oproj(2, planes[2][0], planes[2][1], fill)
            attn_head(3, h, planes[3][0], planes[3][1], 3)
        drain(len(fill))
        # tail: o_proj of chunk 3
        enqueue_oproj(3, planes[3][0], planes[3][1], tailq)
        drain(len(tailq), tailq)


def _build():
    nc = bacc.Bacc("TRN2", target_bir_lowering=False, debug=False,
                   num_devices=NCORES)
    xh = nc.dram_tensor("xh", [128, NE, S], FP8, kind="ExternalInput")
    xl = nc.dram_tensor("xl", [128, NE, S], FP8, kind="ExternalInput")
    wqh = nc.dram_tensor("wqh", [128, NE, GD], FP8, kind="ExternalInput")
    wql = nc.dram_tensor("wql", [128, NE, GD], FP8, kind="ExternalInput")
    wkh = nc.dram_tensor("wkh", [128, NE, D], FP8, kind="ExternalInput")
    wkl = nc.dram_tensor("wkl", [128, NE, D], FP8, kind="ExternalInput")
    wvh = nc.dram_tensor("wvh", [128, NE, D], FP8, kind="ExternalInput")
    wvl = nc.dram_tensor("wvl", [128, NE, D], FP8, kind="ExternalInput")
    woh = nc.dram_tensor("woh", [128, G, E], FP8, kind="ExternalInput")
    wol = nc.dram_tensor("wol", [128, G, E], FP8, kind="ExternalInput")
    cosT = nc.dram_tensor("cosT", [D, S], BF16, kind="ExternalInput")
    sinT = nc.dram_tensor("sinT", [D, S], BF16, kind="ExternalInput")
    rotP = nc.dram_tensor("rotP", [128, 128], BF16, kind="ExternalInput")
    onesb = nc.dram_tensor("onesb", [128, 1], BF16, kind="ExternalInput")
    out = nc.dram_tensor("out", [S, E], BF16, kind="ExternalOutput")
    with tile.TileContext(nc) as tc:
        _emit(nc, tc, xh, xl, wqh, wql, wkh, wkl, wvh, wvl, woh, wol, cosT,
              sinT, rotP, onesb, out)
    nc.compile()
    return nc


def _rope_tables():
    inv = 1.0 / (ROPE_BASE ** (np.arange(0, D, 2, dtype=np.float64) / D))
    t = np.arange(S, dtype=np.float64)
    freqs = t[:, None] * inv[None, :]                    # [S, D/2]
    emb = np.concatenate([freqs, freqs], axis=-1)        # [S, D]
    cosT = np.cos(emb).T.astype(ml_dtypes.bfloat16)      # [D, S]
    sinT = np.sin(emb).T.astype(ml_dtypes.bfloat16)
    return np.ascontiguousarray(cosT), np.ascontiguousarray(sinT)


def _rot_perm():
    # rot(q)[d] = -q[d+64] for d<64, +q[d-64] for d>=64, as a stationary
    # matmul operand: rot = P^T @ q with P[k, m] below.
    p = np.zeros((128, 128), dtype=ml_dtypes.bfloat16)
    for d in range(64):
        p[d + 64, d] = -1.0
        p[d, d + 64] = 1.0
    return p


def _pm(a, nblk):
    """[K, M] -> partition-major [128, nblk, M] (K = nblk*128)."""
    k, m = a.shape
    return np.ascontiguousarray(a.reshape(nblk, 128, m).transpose(1, 0, 2))


_NC = None
LAST_RESULTS = None


def kernel(hidden_states, wq, wk, wv, wo):
    global _NC, LAST_RESULTS
    if _NC is None:
        _NC = _build()
    cosT, sinT = _rope_tables()
    onesb = np.full((128, 1), RED, dtype=ml_dtypes.bfloat16)
    rotP = _rot_perm()
    f8 = ml_dtypes.float8_e4m3

    def planes(a, scale):
        hi = (scale * a).astype(f8)
        lo = (scale * a - hi.astype(np.float32)).astype(f8)
        return hi, lo

    hs = np.asarray(hidden_states, dtype=np.float32)
    wq = np.asarray(wq, dtype=np.float32)
    wk = np.asarray(wk, dtype=np.float32)
    wv = np.asarray(wv, dtype=np.float32)
    wo = np.asarray(wo, dtype=np.float32)
    xplanes = []
    for b in range(B):
        hi, lo = planes(np.ascontiguousarray(hs[b].T), AX)
        xplanes.append((_pm(hi, NE), _pm(lo, NE)))

    in_maps = []
    for core in range(NCORES):
        b, g = divmod(core, G)
        wqh_, wql_ = planes(wq[:, GD * g:GD * (g + 1)], AW)
        wkh_, wkl_ = planes(wk[:, D * g:D * (g + 1)], AW)
        wvh_, wvl_ = planes(wv[:, D * g:D * (g + 1)], AW)
        woh_, wol_ = planes(wo[GD * g:GD * (g + 1), :], AW)
        in_maps.append({
            "xh": xplanes[b][0],
            "xl": xplanes[b][1],
            "wqh": _pm(wqh_, NE),
            "wql": _pm(wql_, NE),
            "wkh": _pm(wkh_, NE),
            "wkl": _pm(wkl_, NE),
            "wvh": _pm(wvh_, NE),
            "wvl": _pm(wvl_, NE),
            "woh": _pm(woh_, G),
            "wol": _pm(wol_, G),
            "cosT": cosT,
            "sinT": sinT,
            "rotP": rotP,
            "onesb": onesb,
        })

    res = run_bass_kernel_spmd(_NC, in_maps, list(range(NCORES)))
    LAST_RESULTS = res
    outs = [np.asarray(res.results[i]["out"], dtype=np.float32)
            for i in range(NCORES)]
    full = np.stack([sum(outs[b * G:(b + 1) * G]) for b in range(B)], axis=0)
    return (full / PSC).astype(np.float32)


# revision 11
# speedup vs baseline: 1.2087x; 1.0004x over previous
"""GQA (16 q-heads / 4 kv-heads, D=128, S=2048, E=2048, B=2) on 8 trn2 cores.

Sharding: core = 4*b + g  (b in {0,1} batch, g in {0..3} kv-head group).
Each core computes its batch's 4 query heads (one kv group) end-to-end and
the host sums the 4 partial o_proj outputs per batch.

v4 (balanced Act/PE, paired exp, direct stores):
  - Host pre-arranges all tensors partition-major so every DMA moves >=512B
    contiguous runs (no strided-transpose DMAs); wq/wk/wv/wo resident in
    SBUF fp8 hi/lo planes, x resident per chunk.
  - V projected directly into natural [keys, D] layout (x-tile stationary,
    wv moving) -- no PE transpose, no staging copy.
  - Attention in four 512-wide query chunks; key tiles processed in PAIRS:
    scores for tiles 2j,2j+1 -> one [128,2,512] PSUM tile -> ONE Act exp
    instruction (halves Act instruction overhead); AV per tile in bf16.
    Pipeline depth 2 pairs: scp(j) + drained fill units run before
    avp(j-2), so exp latency is fully hidden.
  - Deferred projection/o_proj work queued as ~2-matmul units and drained a
    few per pair-iteration: B0<-Q1, B1<-Q2+O(c0), B2<-Q3+O(c1), B3<-O(c2),
    tail<-O(c3).  o_proj accumulates in PSUM and stores PSUM->DRAM direct.
"""

import numpy as np
import ml_dtypes

import concourse.bass as bass
import concourse.bacc as bacc
import concourse.mybir as mybir
import concourse.tile as tile
from concourse.bass_utils import run_bass_kernel_spmd

B, S, E = 2, 2048, 2048
H, HKV, D = 16, 4, 128
G = H // HKV          # 4 query heads per kv group
GD = G * D            # 512 channels per group
NCORES = 8
SCALE = 1.0 / float(np.sqrt(D))
ROPE_BASE = 10000.0
AX = 16.0             # fp8 plane scale for x
AW = 64.0             # fp8 plane scale for wq/wk/wv/wo
PSC = AX * AW         # q/k/v come out scaled by PSC
SCALE_EFF = SCALE / (PSC * PSC)   # folds the q*k scale into exp
AO = 16.0             # fp8 plane scale for the normalized attention output
# the softmax reduce uses (PSC/AO)-valued "ones", so ot = AO * attn_out and
# the o_proj result comes out scaled by AO*AW = PSC; the host divides once.
RED = PSC / AO

NE = E // 128         # 16 e-blocks (contraction for projections)
NC4 = S // 512        # 4 position chunks of 512
NST = S // 128        # 16 sk-tiles of 128
NP = NST // 2         # 8 sk-tile PAIRS

F32 = mybir.dt.float32
BF16 = mybir.dt.bfloat16
FP8 = mybir.dt.float8e4
DR = mybir.MatmulPerfMode.DoubleRow
AF = mybir.ActivationFunctionType
OP = mybir.AluOpType

PLANES = ((0, 0), (0, 1), (1, 0))   # (w_plane, x_plane): HH, HL, LH


def _emit(nc, tc, xh, xl, wqh, wql, wkh, wkl, wvh, wvl, woh, wol, cosT,
          sinT, rotP, onesb, out):
    from contextlib import ExitStack
    import collections
    es = ExitStack()
    with es:
        cpool = es.enter_context(tc.tile_pool(name="const", bufs=1))
        xpool = es.enter_context(tc.tile_pool(name="xs", bufs=1))
        rpool = es.enter_context(tc.tile_pool(name="rope", bufs=2))
        etpool = es.enter_context(tc.tile_pool(name="et", bufs=4))
        bcspool = es.enter_context(tc.tile_pool(name="bcs", bufs=2))
        dnpool = es.enter_context(tc.tile_pool(name="dn", bufs=2))
        rcpool = es.enter_context(tc.tile_pool(name="rc", bufs=2))
        otpool = es.enter_context(tc.tile_pool(name="ot", bufs=2))
        ostgpool = es.enter_context(tc.tile_pool(name="ostg", bufs=6))
        pssc = es.enter_context(
            tc.tile_pool(name="pssc", bufs=2, space=bass.MemorySpace.PSUM))
        psav = es.enter_context(
            tc.tile_pool(name="psav", bufs=1, space=bass.MemorySpace.PSUM))
        psmx = es.enter_context(
            tc.tile_pool(name="psmx", bufs=3, space=bass.MemorySpace.PSUM))

        # ---- persistent SBUF tensors ----
        rp_sb = cpool.tile([128, 128], BF16, tag="rp")
        ones_sb = cpool.tile([128, 1], BF16, tag="ones")
        cos_sb = cpool.tile([D, S], BF16, tag="cos")
        sin_sb = cpool.tile([D, S], BF16, tag="sin")
        wk_t = [cpool.tile([128, NE, D], FP8, tag=f"wkt{i}", name=f"wkt{i}")
                for i in range(2)]
        wv_t = [cpool.tile([128, NE, D], FP8, tag=f"wvt{i}", name=f"wvt{i}")
                for i in range(2)]
        wq_t = [cpool.tile([128, NE, GD], FP8, tag=f"wqt{i}", name=f"wqt{i}")
                for i in range(2)]
        wo_t = [cpool.tile([128, G, E], FP8, tag=f"wot{i}", name=f"wot{i}")
                for i in range(2)]
        kt = cpool.tile([D, S], BF16, tag="kt")
        qt = [cpool.tile([D, S], BF16, tag=f"qt{h}", name=f"qt{h}")
              for h in range(G)]
        vn = cpool.tile([128, NST, D], BF16, tag="vn")

        xt = {}

        def load_x(c4, nsplit):
            for i, t in enumerate((xh, xl)):
                xtile = xpool.tile([128, NE, 512], FP8, tag=f"x{c4}_{i}",
                                   name=f"x{c4}_{i}")
                step = NE // nsplit
                for s in range(nsplit):
                    nc.sync.dma_start(
                        out=xtile[:, s * step:(s + 1) * step, :],
                        in_=t.ap()[:, s * step:(s + 1) * step,
                                   c4 * 512:(c4 + 1) * 512])
                xt[(c4, i)] = xtile

        # ---- DMA schedule (dependency order; contiguous runs >=512B) ----
        for i, t in enumerate((wkh, wkl)):
            nc.sync.dma_start(out=wk_t[i][:], in_=t.ap())
        load_x(0, 4)
        nc.sync.dma_start(out=rp_sb[:], in_=rotP.ap())
        for i, t in enumerate((wvh, wvl)):
            nc.sync.dma_start(out=wv_t[i][:], in_=t.ap())

        def load_cs(c4):
            sl = slice(c4 * 512, (c4 + 1) * 512)
            nc.sync.dma_start(out=cos_sb[:, sl], in_=cosT.ap()[:, sl])
            nc.sync.dma_start(out=sin_sb[:, sl], in_=sinT.ap()[:, sl])

        load_cs(0)
        for i, t in enumerate((wqh, wql)):
            for sp in range(2):
                nc.sync.dma_start(
                    out=wq_t[i][:, sp * 8:(sp + 1) * 8, :],
                    in_=t.ap()[:, sp * 8:(sp + 1) * 8, :])
        load_x(1, 2)
        load_cs(1)
        load_x(2, 2)
        load_cs(2)
        load_x(3, 2)
        load_cs(3)
        nc.sync.dma_start(out=ones_sb[:], in_=onesb.ap())
        for i, t in enumerate((woh, wol)):
            for sp in range(2):
                nc.sync.dma_start(
                    out=wo_t[i][:, sp * 2:(sp + 1) * 2, :],
                    in_=t.ap()[:, sp * 2:(sp + 1) * 2, :])

        # ---- fill-unit queues (labelled; require() force-drains FIFO
        # until a label's units are all emitted -- keeps emission order
        # consistent with data dependencies) ----
        fill = collections.deque()
        tailq = collections.deque()
        pending = collections.Counter()

        def enq(label, fn, q=None):
            (fill if q is None else q).append((label, fn))
            pending[label] += 1

        def drain(n, q=None):
            q = fill if q is None else q
            while n > 0 and q:
                lab, fn = q.popleft()
                fn()
                pending[lab] -= 1
                n -= 1

        def require(label):
            while pending.get(label, 0) > 0:
                lab, fn = fill.popleft()
                fn()
                pending[lab] -= 1

        # ---- rope: rotate_half as signed-permutation matmul ----
        def rope_start(ps, eng):
            qraw = rpool.tile([128, 512], BF16, tag="qraw")
            if eng == 'act':
                nc.scalar.copy(qraw[:], ps[:])
            else:
                nc.vector.tensor_copy(qraw[:], ps[:])
            return qraw

        def rope_finish(dst, qraw, sl):
            tmc = rpool.tile([128, 512], BF16, tag="tmc")
            t2 = rpool.tile([128, 512], BF16, tag="t2")
            rot = psmx.tile([128, 512], F32, tag="mx", name="rot")
            nc.tensor.matmul(rot[:], rp_sb[:], qraw[:], start=True, stop=True)
            nc.gpsimd.tensor_tensor(tmc[:], qraw[:], cos_sb[:, sl], OP.mult)
            nc.vector.tensor_tensor(t2[:], rot[:], sin_sb[:, sl], OP.mult)
            nc.vector.tensor_tensor(dst, tmc[:], t2[:], OP.add)

        # ---- projections (fp8 DoubleRow, 3 quant planes) ----
        def proj_mms(wt, cslice, c4):
            mms = []
            for wi, xi in PLANES:
                for p in range(NE // 2):
                    mms.append((wt[wi][:, 2 * p:2 * p + 2, cslice],
                                xt[(c4, xi)][:, 2 * p:2 * p + 2, :]))
            return mms

        def kproj(c4):
            sl = slice(c4 * 512, (c4 + 1) * 512)
            ps = psmx.tile([128, 512], F32, tag="mx", name="ps")
            mms = proj_mms(wk_t, slice(0, D), c4)
            for i, (wa, xa) in enumerate(mms):
                nc.tensor.matmul(ps[:], wa, xa, perf_mode=DR,
                                 start=(i == 0), stop=(i == len(mms) - 1))
            return ps, sl

        def vproj_mms(c4):
            vp = psmx.tile([128, 4, 128], F32, tag="mx", name="vp")
            for i in range(4):
                ksl = slice(i * 128, (i + 1) * 128)
                j = 0
                for wi, xi in PLANES:
                    for p in range(NE // 2):
                        nc.tensor.matmul(
                            vp[:, i, :],
                            xt[(c4, xi)][:, 2 * p:2 * p + 2, ksl],
                            wv_t[wi][:, 2 * p:2 * p + 2, :],
                            perf_mode=DR, start=(j == 0), stop=(j == 23))
                        j += 1
            return vp

        def vn_copy(c4, vp, eng):
            dst = vn[:, c4 * 4:(c4 + 1) * 4, :]
            if eng == 'act':
                nc.scalar.copy(dst, vp[:])
            elif eng == 'pool':
                nc.gpsimd.tensor_copy(dst, vp[:])
            else:
                nc.vector.tensor_copy(dst, vp[:])

        def qproj(h, c4):
            # phase-A inline Q projection; rope finished by caller interleave
            sl = slice(c4 * 512, (c4 + 1) * 512)
            ps = psmx.tile([128, 512], F32, tag="mx", name="ps")
            mms = proj_mms(wq_t, slice(h * D, (h + 1) * D), c4)
            for i, (wa, xa) in enumerate(mms):
                nc.tensor.matmul(ps[:], wa, xa, perf_mode=DR,
                                 start=(i == 0), stop=(i == len(mms) - 1))
            return ps, sl

        def enqueue_qproj(h, c4):
            sl = slice(c4 * 512, (c4 + 1) * 512)
            lab = f"Q{c4}h{h}"
            state = {}
            nmm = 24

            def mk(j):
                def unit():
                    if j == 0:
                        state['ps'] = psmx.tile([128, 512], F32, tag="mx",
                                                name="ps")
                        state['mms'] = proj_mms(
                            wq_t, slice(h * D, (h + 1) * D), c4)
                    ps = state['ps']
                    for jj in (2 * j, 2 * j + 1):
                        wa, xa = state['mms'][jj]
                        nc.tensor.matmul(ps[:], wa, xa, perf_mode=DR,
                                         start=(jj == 0), stop=(jj == nmm - 1))
                return unit
            for j in range(nmm // 2):
                enq(lab, mk(j))

            def fin():
                qraw = rope_start(state['ps'], 'dve')
                rope_finish(qt[h][:, sl], qraw, sl)
            enq(lab, fin)

        def enqueue_kproj(c4):
            sl = slice(c4 * 512, (c4 + 1) * 512)
            lab = f"K{c4}"
            state = {}
            nmm = 24

            def mk(j):
                def unit():
                    if j == 0:
                        state['ps'] = psmx.tile([128, 512], F32, tag="mx",
                                                name="ps")
                        state['mms'] = proj_mms(wk_t, slice(0, D), c4)
                    ps = state['ps']
                    for jj in (2 * j, 2 * j + 1):
                        wa, xa = state['mms'][jj]
                        nc.tensor.matmul(ps[:], wa, xa, perf_mode=DR,
                                         start=(jj == 0), stop=(jj == nmm - 1))
                return unit
            for j in range(nmm // 2):
                enq(lab, mk(j))

            def fin():
                qraw = rope_start(state['ps'], 'dve')
                rope_finish(kt[:, sl], qraw, sl)
            enq(lab, fin)

        def enqueue_vproj(c4):
            lab = f"V{c4}"
            state = {}

            def mkmm(i, g):
                def unit():
                    if i == 0 and g == 0:
                        state['vp'] = psmx.tile([128, 4, 128], F32, tag="mx",
                                                name="vp")
                    vp = state['vp']
                    ksl = slice(i * 128, (i + 1) * 128)
                    mms = [(xt[(c4, xi)][:, 2 * p:2 * p + 2, ksl],
                            wv_t[wi][:, 2 * p:2 * p + 2, :])
                           for wi, xi in PLANES for p in range(NE // 2)]
                    for jj in range(8 * g, 8 * g + 8):
                        sa, ma = mms[jj]
                        nc.tensor.matmul(vp[:, i, :], sa, ma, perf_mode=DR,
                                         start=(jj == 0), stop=(jj == 23))
                return unit

            def mkcp(i):
                def unit():
                    nc.vector.tensor_copy(vn[:, c4 * 4 + i, :],
                                          state['vp'][:, i, :])
                return unit
            for i in range(4):
                for g in range(3):
                    enq(lab, mkmm(i, g))
                enq(lab, mkcp(i))

        # ---- o_proj: ot (fp8 hi/lo planes) @ wo, PSUM -> bf16 SBUF
        # staging (two 512-col groups share one [128,1024] staging tile and
        # one store) ----
        def enqueue_oproj(ci, oth, otl, q):
            off = ci * 512
            for st in range(4):
                ssl = slice(st * 128, (st + 1) * 128)
                shared = {}
                for eo in range(4):
                    esl = slice(eo * 512, (eo + 1) * 512)
                    state = {}
                    # head-pair i=0 mms first: they only need heads 0/1 of
                    # the ot planes, so the tail can start before the last
                    # head's epilogue lands.
                    mms = []
                    for i in range(2):
                        for src, wi in ((oth, 0), (oth, 1), (otl, 0)):
                            mms.append((src[:, 2 * i:2 * i + 2, ssl],
                                        wo_t[wi][:, 2 * i:2 * i + 2, esl]))

                    def mk(st, eo, j, mms=mms, state=state, shared=shared):
                        def unit():
                            if j == 0:
                                state['op'] = psmx.tile([128, 512], F32,
                                                        tag="mx", name="op")
                            op = state['op']
                            for jj in (2 * j, 2 * j + 1):
                                oa, wa = mms[jj]
                                nc.tensor.matmul(op[:], oa, wa, perf_mode=DR,
                                                 start=(jj == 0),
                                                 stop=(jj == 5))
                            if j == 2:
                                half = eo % 2
                                if half == 0:
                                    shared['ostg'] = ostgpool.tile(
                                        [128, 1024], BF16, tag="ostg",
                                        name="ostg")
                                ostg = shared['ostg']
                                dst = ostg[:, half * 512:(half + 1) * 512]
                                # GPSIMD cannot read PSUM: alternate Act/DVE
                                if (st + eo // 2) % 2 == 0:
                                    nc.scalar.copy(dst, op[:])
                                else:
                                    nc.vector.tensor_copy(dst, op[:])
                                if half == 1:
                                    nc.sync.dma_start(
                                        out=out.ap()[off + st * 128:
                                                     off + (st + 1) * 128,
                                                     (eo - 1) * 512:
                                                     (eo + 1) * 512],
                                        in_=ostg[:])
                        return unit
                    for j in range(3):
                        enq(f"O{ci}", mk(st, eo, j), q)

        # ---- K/V as fill units (not used in final schedule; kept simple) --

        # ---- attention: paired key tiles, depth-2 pair pipeline ----
        def attn_head(ci, h, oth, otl, drain_n):
            off = ci * 512
            qsl = slice(off, off + 512)
            require(f"Q{ci}h{h}")
            dn = dnpool.tile([128, 512], BF16, tag="dn")
            av = psav.tile([D, 512], F32, tag="av")

            def scp_exp(j):
                require(f"K{(2 * j + 1) // 4}")
                sc = pssc.tile([128, 2, 512], F32, tag="sc")
                for tt in range(2):
                    t = 2 * j + tt
                    nc.tensor.matmul(sc[:, tt, :],
                                     kt[:, t * 128:(t + 1) * 128],
                                     qt[h][:, qsl], start=True, stop=True)
                et = etpool.tile([128, 2, 512], BF16, tag="et")
                nc.scalar.activation(et[:], sc[:], AF.Exp, scale=SCALE_EFF)
                return et

            def avp(j, et):
                require(f"V{(2 * j + 1) // 4}")
                for tt in range(2):
                    t = 2 * j + tt
                    nc.tensor.matmul(av[:], vn[:, t, :], et[:, tt, :],
                                     start=(t == 0), stop=(t == NST - 1))

            def dnp(j, et):
                for tt in range(2):
                    if j == 0 and tt == 0:
                        nc.vector.tensor_copy(dn[:], et[:, 0, :])
                    else:
                        nc.vector.tensor_tensor(dn[:], dn[:], et[:, tt, :],
                                                OP.add)

            ets = {}
            ets[0] = scp_exp(0)
            ets[1] = scp_exp(1)
            drain(drain_n)
            for j in range(2, NP):
                ets[j] = scp_exp(j)
                drain(drain_n)
                avp(j - 2, ets[j - 2])
                dnp(j - 2, ets[j - 2])
                del ets[j - 2]
            drain(drain_n)
            avp(NP - 2, ets[NP - 2])
            dnp(NP - 2, ets[NP - 2])
            drain(drain_n)
            avp(NP - 1, ets[NP - 1])
            dnp(NP - 1, ets[NP - 1])
            drain(4)
            # epilogue: denominator reduce, reciprocal, normalize, fp8 planes
            sm = psmx.tile([1, 512], F32, tag="mx", name="sm")
            nc.tensor.matmul(sm[:], ones_sb[:, 0:1], dn[:],
                             start=True, stop=True)
            rc = rcpool.tile([1, 512], BF16, tag="rc")
            with nc.allow_low_precision(reason="bf16 softmax denom recip"):
                nc.vector.reciprocal(rc[:], sm[:])
            bcs = bcspool.tile([128, 512], BF16, tag="bcs")
            nc.gpsimd.partition_broadcast(bcs[:], rc[:])
            otf = rpool.tile([D, 512], F32, tag="otf")
            nc.vector.tensor_tensor(otf[:], av[:], bcs[:], OP.mult)
            nc.scalar.copy(oth[:, h, :], otf[:])
            nc.vector.tensor_tensor(otl[:, h, :], otf[:], oth[:, h, :],
                                    OP.subtract)

        # ====== phase A: K/V chunks 0-1, Q chunk 0; rest drains into B0 =====
        ps, sl = kproj(0)
        qraw = rope_start(ps, 'act')
        vp = vproj_mms(0)
        rope_finish(kt[:, sl], qraw, sl)
        vn_copy(0, vp, 'act')
        prev = None
        for h in range(G):
            ps, sl = qproj(h, 0)
            if prev is not None:
                ph, pq, psl = prev
                rope_finish(qt[ph][:, psl], pq, psl)
            qraw = rope_start(ps, 'act')
            prev = (h, qraw, sl)
        ps, sl = kproj(1)
        ph, pq, psl = prev
        rope_finish(qt[ph][:, psl], pq, psl)
        qraw = rope_start(ps, 'act')
        vp = vproj_mms(1)
        rope_finish(kt[:, sl], qraw, sl)
        vn_copy(1, vp, 'act')

        # ================= B windows: attention + drained fills =============
        def ot_planes(ci):
            hi = otpool.tile([128, G, 512], FP8, tag="oth", name=f"oth{ci}")
            lo = otpool.tile([128, G, 512], FP8, tag="otl", name=f"otl{ci}")
            return hi, lo

        planes = {}
        # B0: fills = K/V chunks 2,3 (ledger-paced), then Q chunk 1
        planes[0] = ot_planes(0)
        enqueue_kproj(2)
        enqueue_vproj(2)
        enqueue_kproj(3)
        enqueue_vproj(3)
        for hq in range(G):
            enqueue_qproj(hq, 1)
        for h in range(G):
            attn_head(0, h, planes[0][0], planes[0][1], 6 if h == 0 else 3)
        # B1: fills += Q chunk 2 + o_proj of chunk 0
        planes[1] = ot_planes(1)
        for h in range(G):
            if h == 0:
                for hq in range(G):
                    enqueue_qproj(hq, 2)
            elif h == 1:
                enqueue_oproj(0, planes[0][0], planes[0][1], fill)
            attn_head(1, h, planes[1][0], planes[1][1], 3)
        # B2: fills += Q chunk 3 + o_proj of chunk 1
        planes[2] = ot_planes(2)
        for h in range(G):
            if h == 0:
                for hq in range(G):
                    enqueue_qproj(hq, 3)
            elif h == 1:
                enqueue_oproj(1, planes[1][0], planes[1][1], fill)
            attn_head(2, h, planes[2][0], planes[2][1], 3)
        # B3: fills += o_proj of chunk 2
        planes[3] = ot_planes(3)
        for h in range(G):
            if h == 0:
                enqueue_oproj(2, planes[2][0], planes[2][1], fill)
            attn_head(3, h, planes[3][0], planes[3][1], 3)
        drain(len(fill))
        # tail: o_proj of chunk 3
        enqueue_oproj(3, planes[3][0], planes[3][1], tailq)
        drain(len(tailq), tailq)


def _build():
    nc = bacc.Bacc("TRN2", target_bir_lowering=False, debug=False,
                   num_devices=NCORES)
    xh = nc.dram_tensor("xh", [128, NE, S], FP8, kind="ExternalInput")
    xl = nc.dram_tensor("xl", [128, NE, S], FP8, kind="ExternalInput")
    wqh = nc.dram_tensor("wqh", [128, NE, GD], FP8, kind="ExternalInput")
    wql = nc.dram_tensor("wql", [128, NE, GD], FP8, kind="ExternalInput")
    wkh = nc.dram_tensor("wkh", [128, NE, D], FP8, kind="ExternalInput")
    wkl = nc.dram_tensor("wkl", [128, NE, D], FP8, kind="ExternalInput")
    wvh = nc.dram_tensor("wvh", [128, NE, D], FP8, kind="ExternalInput")
    wvl = nc.dram_tensor("wvl", [128, NE, D], FP8, kind="ExternalInput")
    woh = nc.dram_tensor("woh", [128, G, E], FP8, kind="ExternalInput")
    wol = nc.dram_tensor("wol", [128, G, E], FP8, kind="ExternalInput")
    cosT = nc.dram_tensor("cosT", [D, S], BF16, kind="ExternalInput")
    sinT = nc.dram_tensor("sinT", [D, S], BF16, kind="ExternalInput")
    rotP = nc.dram_tensor("rotP", [128, 128], BF16, kind="ExternalInput")
    onesb = nc.dram_tensor("onesb", [128, 1], BF16, kind="ExternalInput")
    out = nc.dram_tensor("out", [S, E], BF16, kind="ExternalOutput")
    with tile.TileContext(nc) as tc:
        _emit(nc, tc, xh, xl, wqh, wql, wkh, wkl, wvh, wvl, woh, wol, cosT,
              sinT, rotP, onesb, out)
    nc.compile()
    return nc


def _rope_tables():
    inv = 1.0 / (ROPE_BASE ** (np.arange(0, D, 2, dtype=np.float64) / D))
    t = np.arange(S, dtype=np.float64)
    freqs = t[:, None] * inv[None, :]                    # [S, D/2]
    emb = np.concatenate([freqs, freqs], axis=-1)        # [S, D]
    cosT = np.cos(emb).T.astype(ml_dtypes.bfloat16)      # [D, S]
    sinT = np.sin(emb).T.astype(ml_dtypes.bfloat16)
    return np.ascontiguousarray(cosT), np.ascontiguousarray(sinT)


def _rot_perm():
    # rot(q)[d] = -q[d+64] for d<64, +q[d-64] for d>=64, as a stationary
    # matmul operand: rot = P^T @ q with P[k, m] below.
    p = np.zeros((128, 128), dtype=ml_dtypes.bfloat16)
    for d in range(64):
        p[d + 64, d] = -1.0
        p[d, d + 64] = 1.0
    return p


def _pm(a, nblk):
    """[K, M] -> partition-major [128, nblk, M] (K = nblk*128)."""
    k, m = a.shape
    return np.ascontiguousarray(a.reshape(nblk, 128, m).transpose(1, 0, 2))


_NC = None
LAST_RESULTS = None


def kernel(hidden_states, wq, wk, wv, wo):
    global _NC, LAST_RESULTS
    if _NC is None:
        _NC = _build()
    cosT, sinT = _rope_tables()
    onesb = np.full((128, 1), RED, dtype=ml_dtypes.bfloat16)
    rotP = _rot_perm()
    f8 = ml_dtypes.float8_e4m3

    def planes(a, scale):
        hi = (scale * a).astype(f8)
        lo = (scale * a - hi.astype(np.float32)).astype(f8)
        return hi, lo

    hs = np.asarray(hidden_states, dtype=np.float32)
    wq = np.asarray(wq, dtype=np.float32)
    wk = np.asarray(wk, dtype=np.float32)
    wv = np.asarray(wv, dtype=np.float32)
    wo = np.asarray(wo, dtype=np.float32)
    xplanes = []
    for b in range(B):
        hi, lo = planes(np.ascontiguousarray(hs[b].T), AX)
        xplanes.append((_pm(hi, NE), _pm(lo, NE)))

    in_maps = []
    for core in range(NCORES):
        b, g = divmod(core, G)
        wqh_, wql_ = planes(wq[:, GD * g:GD * (g + 1)], AW)
        wkh_, wkl_ = planes(wk[:, D * g:D * (g + 1)], AW)
        wvh_, wvl_ = planes(wv[:, D * g:D * (g + 1)], AW)
        woh_, wol_ = planes(wo[GD * g:GD * (g + 1), :], AW)
        in_maps.append({
            "xh": xplanes[b][0],
            "xl": xplanes[b][1],
            "wqh": _pm(wqh_, NE),
            "wql": _pm(wql_, NE),
            "wkh": _pm(wkh_, NE),
            "wkl": _pm(wkl_, NE),
            "wvh": _pm(wvh_, NE),
            "wvl": _pm(wvl_, NE),
            "woh": _pm(woh_, G),
            "wol": _pm(wol_, G),
            "cosT": cosT,
            "sinT": sinT,
            "rotP": rotP,
            "onesb": onesb,
        })

    res = run_bass_kernel_spmd(_NC, in_maps, list(range(NCORES)))
    LAST_RESULTS = res
    outs = [np.asarray(res.results[i]["out"], dtype=np.float32)
            for i in range(NCORES)]
    full = np.stack([sum(outs[b * G:(b + 1) * G]) for b in range(B)], axis=0)
    return (full / PSC).astype(np.float32)


# revision 12
# speedup vs baseline: 1.2243x; 1.0130x over previous
"""GQA (16 q-heads / 4 kv-heads, D=128, S=2048, E=2048, B=2) on 8 trn2 cores.

Sharding: core = 4*b + g  (b in {0,1} batch, g in {0..3} kv-head group).
Each core computes its batch's 4 query heads (one kv group) end-to-end and
the host sums the 4 partial o_proj outputs per batch.

v4 (balanced Act/PE, paired exp, direct stores):
  - Host pre-arranges all tensors partition-major so every DMA moves >=512B
    contiguous runs (no strided-transpose DMAs); wq/wk/wv/wo resident in
    SBUF fp8 hi/lo planes, x resident per chunk.
  - V projected directly into natural [keys, D] layout (x-tile stationary,
    wv moving) -- no PE transpose, no staging copy.
  - Attention in four 512-wide query chunks; key tiles processed in PAIRS:
    scores for tiles 2j,2j+1 -> one [128,2,512] PSUM tile -> ONE Act exp
    instruction (halves Act instruction overhead); AV per tile in bf16.
    Pipeline depth 2 pairs: scp(j) + drained fill units run before
    avp(j-2), so exp latency is fully hidden.
  - Deferred projection/o_proj work queued as ~2-matmul units and drained a
    few per pair-iteration: B0<-Q1, B1<-Q2+O(c0), B2<-Q3+O(c1), B3<-O(c2),
    tail<-O(c3).  o_proj accumulates in PSUM and stores PSUM->DRAM direct.
"""

import numpy as np
import ml_dtypes

import concourse.bass as bass
import concourse.bacc as bacc
import concourse.mybir as mybir
import concourse.tile as tile
from concourse.bass_utils import run_bass_kernel_spmd

B, S, E = 2, 2048, 2048
H, HKV, D = 16, 4, 128
G = H // HKV          # 4 query heads per kv group
GD = G * D            # 512 channels per group
NCORES = 8
SCALE = 1.0 / float(np.sqrt(D))
ROPE_BASE = 10000.0
AX = 16.0             # fp8 plane scale for x
AW = 64.0             # fp8 plane scale for wq/wk/wv/wo
PSC = AX * AW         # q/k/v come out scaled by PSC
SCALE_EFF = SCALE / (PSC * PSC)   # folds the q*k scale into exp
AO = 16.0             # fp8 plane scale for the normalized attention output
# the softmax reduce uses (PSC/AO)-valued "ones", so ot = AO * attn_out and
# the o_proj result comes out scaled by AO*AW = PSC; the host divides once.
RED = PSC / AO

NE = E // 128         # 16 e-blocks (contraction for projections)
NC4 = S // 512        # 4 position chunks of 512
NST = S // 128        # 16 sk-tiles of 128
NP = NST // 2         # 8 sk-tile PAIRS

F32 = mybir.dt.float32
BF16 = mybir.dt.bfloat16
FP8 = mybir.dt.float8e4
DR = mybir.MatmulPerfMode.DoubleRow
AF = mybir.ActivationFunctionType
OP = mybir.AluOpType

PLANES = ((0, 0), (0, 1), (1, 0))   # (w_plane, x_plane): HH, HL, LH


def _emit(nc, tc, xh, xl, wqh, wql, wkh, wkl, wvh, wvl, woh, wol, cosT,
          sinT, rotP, onesb, out):
    from contextlib import ExitStack
    import collections
    es = ExitStack()
    with es:
        cpool = es.enter_context(tc.tile_pool(name="const", bufs=1))
        xpool = es.enter_context(tc.tile_pool(name="xs", bufs=1))
        rpool = es.enter_context(tc.tile_pool(name="rope", bufs=2))
        etpool = es.enter_context(tc.tile_pool(name="et", bufs=4))
        bcspool = es.enter_context(tc.tile_pool(name="bcs", bufs=2))
        dnpool = es.enter_context(tc.tile_pool(name="dn", bufs=2))
        rcpool = es.enter_context(tc.tile_pool(name="rc", bufs=2))
        otpool = es.enter_context(tc.tile_pool(name="ot", bufs=2))
        ostgpool = es.enter_context(tc.tile_pool(name="ostg", bufs=6))
        pssc = es.enter_context(
            tc.tile_pool(name="pssc", bufs=2, space=bass.MemorySpace.PSUM))
        psav = es.enter_context(
            tc.tile_pool(name="psav", bufs=1, space=bass.MemorySpace.PSUM))
        psmx = es.enter_context(
            tc.tile_pool(name="psmx", bufs=3, space=bass.MemorySpace.PSUM))

        # ---- persistent SBUF tensors ----
        rp_sb = cpool.tile([128, 128], BF16, tag="rp")
        ones_sb = cpool.tile([128, 1], BF16, tag="ones")
        cos_sb = cpool.tile([D, S], BF16, tag="cos")
        sin_sb = cpool.tile([D, S], BF16, tag="sin")
        wk_t = [cpool.tile([128, NE, D], FP8, tag=f"wkt{i}", name=f"wkt{i}")
                for i in range(2)]
        wv_t = [cpool.tile([128, NE, D], FP8, tag=f"wvt{i}", name=f"wvt{i}")
                for i in range(2)]
        wq_t = [cpool.tile([128, NE, GD], FP8, tag=f"wqt{i}", name=f"wqt{i}")
                for i in range(2)]
        wo_t = [cpool.tile([128, G, E], FP8, tag=f"wot{i}", name=f"wot{i}")
                for i in range(2)]
        kt = cpool.tile([D, S], BF16, tag="kt")
        qt = [cpool.tile([D, S], BF16, tag=f"qt{h}", name=f"qt{h}")
              for h in range(G)]
        vn = cpool.tile([128, NST, D], BF16, tag="vn")

        xt = {}

        def load_x(c4, nsplit):
            for i, t in enumerate((xh, xl)):
                xtile = xpool.tile([128, NE, 512], FP8, tag=f"x{c4}_{i}",
                                   name=f"x{c4}_{i}")
                step = NE // nsplit
                for s in range(nsplit):
                    nc.sync.dma_start(
                        out=xtile[:, s * step:(s + 1) * step, :],
                        in_=t.ap()[:, s * step:(s + 1) * step,
                                   c4 * 512:(c4 + 1) * 512])
                xt[(c4, i)] = xtile

        # ---- DMA schedule (dependency order; contiguous runs >=512B) ----
        for i, t in enumerate((wkh, wkl)):
            nc.sync.dma_start(out=wk_t[i][:], in_=t.ap())
        load_x(0, 4)
        nc.sync.dma_start(out=rp_sb[:], in_=rotP.ap())
        for i, t in enumerate((wvh, wvl)):
            nc.sync.dma_start(out=wv_t[i][:], in_=t.ap())

        def load_cs(c4):
            sl = slice(c4 * 512, (c4 + 1) * 512)
            nc.sync.dma_start(out=cos_sb[:, sl], in_=cosT.ap()[:, sl])
            nc.sync.dma_start(out=sin_sb[:, sl], in_=sinT.ap()[:, sl])

        load_cs(0)
        for i, t in enumerate((wqh, wql)):
            for sp in range(2):
                nc.sync.dma_start(
                    out=wq_t[i][:, sp * 8:(sp + 1) * 8, :],
                    in_=t.ap()[:, sp * 8:(sp + 1) * 8, :])
        load_x(1, 2)
        load_cs(1)
        load_x(2, 2)
        load_cs(2)
        load_x(3, 2)
        load_cs(3)
        nc.sync.dma_start(out=ones_sb[:], in_=onesb.ap())
        for i, t in enumerate((woh, wol)):
            for sp in range(2):
                nc.sync.dma_start(
                    out=wo_t[i][:, sp * 2:(sp + 1) * 2, :],
                    in_=t.ap()[:, sp * 2:(sp + 1) * 2, :])

        # ---- fill-unit queues (labelled; require() force-drains FIFO
        # until a label's units are all emitted -- keeps emission order
        # consistent with data dependencies) ----
        fill = collections.deque()
        tailq = collections.deque()
        pending = collections.Counter()

        def enq(label, fn, q=None):
            (fill if q is None else q).append((label, fn))
            pending[label] += 1

        def drain(n, q=None):
            q = fill if q is None else q
            while n > 0 and q:
                lab, fn = q.popleft()
                fn()
                pending[lab] -= 1
                n -= 1

        def require(label):
            while pending.get(label, 0) > 0:
                lab, fn = fill.popleft()
                fn()
                pending[lab] -= 1

        # ---- rope: rotate_half as signed-permutation matmul ----
        def rope_start(ps, eng):
            qraw = rpool.tile([128, 512], BF16, tag="qraw")
            if eng == 'act':
                nc.scalar.copy(qraw[:], ps[:])
            else:
                nc.vector.tensor_copy(qraw[:], ps[:])
            return qraw

        def rope_finish(dst, qraw, sl):
            tmc = rpool.tile([128, 512], BF16, tag="tmc")
            t2 = rpool.tile([128, 512], BF16, tag="t2")
            rot = psmx.tile([128, 512], F32, tag="mx", name="rot")
            nc.tensor.matmul(rot[:], rp_sb[:], qraw[:], start=True, stop=True)
            nc.gpsimd.tensor_tensor(tmc[:], qraw[:], cos_sb[:, sl], OP.mult)
            nc.vector.tensor_tensor(t2[:], rot[:], sin_sb[:, sl], OP.mult)
            nc.vector.tensor_tensor(dst, tmc[:], t2[:], OP.add)

        # ---- projections (fp8 DoubleRow, 3 quant planes) ----
        def proj_mms(wt, cslice, c4):
            mms = []
            for wi, xi in PLANES:
                for p in range(NE // 2):
                    mms.append((wt[wi][:, 2 * p:2 * p + 2, cslice],
                                xt[(c4, xi)][:, 2 * p:2 * p + 2, :]))
            return mms

        def kproj(c4):
            sl = slice(c4 * 512, (c4 + 1) * 512)
            ps = psmx.tile([128, 512], F32, tag="mx", name="ps")
            mms = proj_mms(wk_t, slice(0, D), c4)
            for i, (wa, xa) in enumerate(mms):
                nc.tensor.matmul(ps[:], wa, xa, perf_mode=DR,
                                 start=(i == 0), stop=(i == len(mms) - 1))
            return ps, sl

        def vproj_mms(c4):
            vp = psmx.tile([128, 4, 128], F32, tag="mx", name="vp")
            for i in range(4):
                ksl = slice(i * 128, (i + 1) * 128)
                j = 0
                for wi, xi in PLANES:
                    for p in range(NE // 2):
                        nc.tensor.matmul(
                            vp[:, i, :],
                            xt[(c4, xi)][:, 2 * p:2 * p + 2, ksl],
                            wv_t[wi][:, 2 * p:2 * p + 2, :],
                            perf_mode=DR, start=(j == 0), stop=(j == 23))
                        j += 1
            return vp

        def vn_copy(c4, vp, eng):
            dst = vn[:, c4 * 4:(c4 + 1) * 4, :]
            if eng == 'act':
                nc.scalar.copy(dst, vp[:])
            elif eng == 'pool':
                nc.gpsimd.tensor_copy(dst, vp[:])
            else:
                nc.vector.tensor_copy(dst, vp[:])

        def qproj(h, c4):
            # phase-A inline Q projection; rope finished by caller interleave
            sl = slice(c4 * 512, (c4 + 1) * 512)
            ps = psmx.tile([128, 512], F32, tag="mx", name="ps")
            mms = proj_mms(wq_t, slice(h * D, (h + 1) * D), c4)
            for i, (wa, xa) in enumerate(mms):
                nc.tensor.matmul(ps[:], wa, xa, perf_mode=DR,
                                 start=(i == 0), stop=(i == len(mms) - 1))
            return ps, sl

        def enqueue_qproj(h, c4):
            sl = slice(c4 * 512, (c4 + 1) * 512)
            lab = f"Q{c4}h{h}"
            state = {}
            nmm = 24

            def mk(j):
                def unit():
                    if j == 0:
                        state['ps'] = psmx.tile([128, 512], F32, tag="mx",
                                                name="ps")
                        state['mms'] = proj_mms(
                            wq_t, slice(h * D, (h + 1) * D), c4)
                    ps = state['ps']
                    for jj in (2 * j, 2 * j + 1):
                        wa, xa = state['mms'][jj]
                        nc.tensor.matmul(ps[:], wa, xa, perf_mode=DR,
                                         start=(jj == 0), stop=(jj == nmm - 1))
                return unit
            for j in range(nmm // 2):
                enq(lab, mk(j))

            def fin():
                qraw = rope_start(state['ps'], 'dve')
                rope_finish(qt[h][:, sl], qraw, sl)
            enq(lab, fin)

        def enqueue_kproj(c4):
            sl = slice(c4 * 512, (c4 + 1) * 512)
            lab = f"K{c4}"
            state = {}
            nmm = 24

            def mk(j):
                def unit():
                    if j == 0:
                        state['ps'] = psmx.tile([128, 512], F32, tag="mx",
                                                name="ps")
                        state['mms'] = proj_mms(wk_t, slice(0, D), c4)
                    ps = state['ps']
                    for jj in (2 * j, 2 * j + 1):
                        wa, xa = state['mms'][jj]
                        nc.tensor.matmul(ps[:], wa, xa, perf_mode=DR,
                                         start=(jj == 0), stop=(jj == nmm - 1))
                return unit
            for j in range(nmm // 2):
                enq(lab, mk(j))

            def fin():
                qraw = rope_start(state['ps'], 'dve')
                rope_finish(kt[:, sl], qraw, sl)
            enq(lab, fin)

        def enqueue_vproj(c4):
            lab = f"V{c4}"
            state = {}

            def mkmm(i, g):
                def unit():
                    if i == 0 and g == 0:
                        state['vp'] = psmx.tile([128, 4, 128], F32, tag="mx",
                                                name="vp")
                    vp = state['vp']
                    ksl = slice(i * 128, (i + 1) * 128)
                    mms = [(xt[(c4, xi)][:, 2 * p:2 * p + 2, ksl],
                            wv_t[wi][:, 2 * p:2 * p + 2, :])
                           for wi, xi in PLANES for p in range(NE // 2)]
                    for jj in range(8 * g, 8 * g + 8):
                        sa, ma = mms[jj]
                        nc.tensor.matmul(vp[:, i, :], sa, ma, perf_mode=DR,
                                         start=(jj == 0), stop=(jj == 23))
                return unit

            def mkcp(i):
                def unit():
                    nc.vector.tensor_copy(vn[:, c4 * 4 + i, :],
                                          state['vp'][:, i, :])
                return unit
            for i in range(4):
                for g in range(3):
                    enq(lab, mkmm(i, g))
                enq(lab, mkcp(i))

        # ---- o_proj: ot (fp8 hi/lo planes) @ wo, PSUM -> bf16 SBUF
        # staging (two 512-col groups share one [128,1024] staging tile and
        # one store) ----
        def enqueue_oproj(ci, oth, otl, q):
            off = ci * 512
            for st in range(4):
                ssl = slice(st * 128, (st + 1) * 128)
                shared = {}
                for eo in range(4):
                    esl = slice(eo * 512, (eo + 1) * 512)
                    state = {}
                    # head-pair i=0 mms first: they only need heads 0/1 of
                    # the ot planes, so the tail can start before the last
                    # head's epilogue lands.
                    mms = []
                    for i in range(2):
                        for src, wi in ((oth, 0), (oth, 1), (otl, 0)):
                            mms.append((src[:, 2 * i:2 * i + 2, ssl],
                                        wo_t[wi][:, 2 * i:2 * i + 2, esl]))

                    def mk(st, eo, j, mms=mms, state=state, shared=shared):
                        def unit():
                            if j == 0:
                                state['op'] = psmx.tile([128, 512], F32,
                                                        tag="mx", name="op")
                            op = state['op']
                            for jj in (2 * j, 2 * j + 1):
                                oa, wa = mms[jj]
                                nc.tensor.matmul(op[:], oa, wa, perf_mode=DR,
                                                 start=(jj == 0),
                                                 stop=(jj == 5))
                            if j == 2:
                                half = eo % 2
                                if half == 0:
                                    shared['ostg'] = ostgpool.tile(
                                        [128, 1024], BF16, tag="ostg",
                                        name="ostg")
                                ostg = shared['ostg']
                                dst = ostg[:, half * 512:(half + 1) * 512]
                                # GPSIMD cannot read PSUM: alternate Act/DVE
                                if (st + eo // 2) % 2 == 0:
                                    nc.scalar.copy(dst, op[:])
                                else:
                                    nc.vector.tensor_copy(dst, op[:])
                                if half == 1:
                                    nc.sync.dma_start(
                                        out=out.ap()[off + st * 128:
                                                     off + (st + 1) * 128,
                                                     (eo - 1) * 512:
                                                     (eo + 1) * 512],
                                        in_=ostg[:])
                        return unit
                    for j in range(3):
                        enq(f"O{ci}", mk(st, eo, j), q)

        # ---- K/V as fill units (not used in final schedule; kept simple) --

        # ---- attention: paired key tiles, depth-2 pair pipeline ----
        def attn_head(ci, h, oth, otl, drain_n):
            off = ci * 512
            qsl = slice(off, off + 512)
            require(f"Q{ci}h{h}")
            dn = dnpool.tile([128, 512], BF16, tag="dn")
            av = psav.tile([D, 512], F32, tag="av")

            def scp_exp(j):
                require(f"K{(2 * j + 1) // 4}")
                sc = pssc.tile([128, 2, 512], F32, tag="sc")
                for tt in range(2):
                    t = 2 * j + tt
                    nc.tensor.matmul(sc[:, tt, :],
                                     kt[:, t * 128:(t + 1) * 128],
                                     qt[h][:, qsl], start=True, stop=True)
                et = etpool.tile([128, 2, 512], BF16, tag="et")
                nc.scalar.activation(et[:], sc[:], AF.Exp, scale=SCALE_EFF)
                return et

            def avp(j, et):
                require(f"V{(2 * j + 1) // 4}")
                for tt in range(2):
                    t = 2 * j + tt
                    nc.tensor.matmul(av[:], vn[:, t, :], et[:, tt, :],
                                     start=(t == 0), stop=(t == NST - 1))

            def dnp(j, et):
                for tt in range(2):
                    if j == 0 and tt == 0:
                        nc.vector.tensor_copy(dn[:], et[:, 0, :])
                    else:
                        nc.vector.tensor_tensor(dn[:], dn[:], et[:, tt, :],
                                                OP.add)

            ets = {}
            ets[0] = scp_exp(0)
            ets[1] = scp_exp(1)
            drain(drain_n)
            for j in range(2, NP):
                ets[j] = scp_exp(j)
                drain(drain_n)
                avp(j - 2, ets[j - 2])
                dnp(j - 2, ets[j - 2])
                del ets[j - 2]
            drain(drain_n)
            avp(NP - 2, ets[NP - 2])
            dnp(NP - 2, ets[NP - 2])
            drain(drain_n)
            avp(NP - 1, ets[NP - 1])
            dnp(NP - 1, ets[NP - 1])
            drain(4)
            # epilogue: denominator reduce, reciprocal, normalize, fp8 planes
            sm = psmx.tile([1, 512], F32, tag="mx", name="sm")
            nc.tensor.matmul(sm[:], ones_sb[:, 0:1], dn[:],
                             start=True, stop=True)
            rc = rcpool.tile([1, 512], BF16, tag="rc")
            with nc.allow_low_precision(reason="bf16 softmax denom recip"):
                nc.vector.reciprocal(rc[:], sm[:])
            bcs = bcspool.tile([128, 512], BF16, tag="bcs")
            nc.gpsimd.partition_broadcast(bcs[:], rc[:])
            otf = rpool.tile([D, 512], F32, tag="otf")
            nc.vector.tensor_tensor(otf[:], av[:], bcs[:], OP.mult)
            nc.gpsimd.tensor_copy(oth[:, h, :], otf[:])
            nc.vector.tensor_tensor(otl[:, h, :], otf[:], oth[:, h, :],
                                    OP.subtract)

        # ====== phase A: K/V chunks 0-1, Q chunk 0; rest drains into B0 =====
        ps, sl = kproj(0)
        qraw = rope_start(ps, 'act')
        vp = vproj_mms(0)
        rope_finish(kt[:, sl], qraw, sl)
        vn_copy(0, vp, 'act')
        prev = None
        for h in range(G):
            ps, sl = qproj(h, 0)
            if prev is not None:
                ph, pq, psl = prev
                rope_finish(qt[ph][:, psl], pq, psl)
            qraw = rope_start(ps, 'act')
            prev = (h, qraw, sl)
        ps, sl = kproj(1)
        ph, pq, psl = prev
        rope_finish(qt[ph][:, psl], pq, psl)
        qraw = rope_start(ps, 'act')
        vp = vproj_mms(1)
        rope_finish(kt[:, sl], qraw, sl)
        vn_copy(1, vp, 'act')

        # ================= B windows: attention + drained fills =============
        def ot_planes(ci):
            hi = otpool.tile([128, G, 512], FP8, tag="oth", name=f"oth{ci}")
            lo = otpool.tile([128, G, 512], FP8, tag="otl", name=f"otl{ci}")
            return hi, lo

        planes = {}
        # B0: fills = K/V chunks 2,3 (ledger-paced), then Q chunk 1
        planes[0] = ot_planes(0)
        enqueue_kproj(2)
        enqueue_vproj(2)
        enqueue_kproj(3)
        enqueue_vproj(3)
        for hq in range(G):
            enqueue_qproj(hq, 1)
        for h in range(G):
            attn_head(0, h, planes[0][0], planes[0][1], 6 if h == 0 else 2)
        # B1: fills += Q chunk 2 + o_proj of chunk 0
        planes[1] = ot_planes(1)
        for h in range(G):
            if h == 0:
                for hq in range(G):
                    enqueue_qproj(hq, 2)
            elif h == 1:
                enqueue_oproj(0, planes[0][0], planes[0][1], fill)
            attn_head(1, h, planes[1][0], planes[1][1], 2)
        # B2: fills += Q chunk 3 + o_proj of chunk 1
        planes[2] = ot_planes(2)
        for h in range(G):
            if h == 0:
                for hq in range(G):
                    enqueue_qproj(hq, 3)
            elif h == 1:
                enqueue_oproj(1, planes[1][0], planes[1][1], fill)
            attn_head(2, h, planes[2][0], planes[2][1], 2)
        # B3: fills += o_proj of chunk 2
        planes[3] = ot_planes(3)
        for h in range(G):
            if h == 0:
                enqueue_oproj(2, planes[2][0], planes[2][1], fill)
            attn_head(3, h, planes[3][0], planes[3][1], 2)
        drain(len(fill))
        # tail: o_proj of chunk 3
        enqueue_oproj(3, planes[3][0], planes[3][1], tailq)
        drain(len(tailq), tailq)


def _build():
    nc = bacc.Bacc("TRN2", target_bir_lowering=False, debug=False,
                   num_devices=NCORES)
    xh = nc.dram_tensor("xh", [128, NE, S], FP8, kind="ExternalInput")
    xl = nc.dram_tensor("xl", [128, NE, S], FP8, kind="ExternalInput")
    wqh = nc.dram_tensor("wqh", [128, NE, GD], FP8, kind="ExternalInput")
    wql = nc.dram_tensor("wql", [128, NE, GD], FP8, kind="ExternalInput")
    wkh = nc.dram_tensor("wkh", [128, NE, D], FP8, kind="ExternalInput")
    wkl = nc.dram_tensor("wkl", [128, NE, D], FP8, kind="ExternalInput")
    wvh = nc.dram_tensor("wvh", [128, NE, D], FP8, kind="ExternalInput")
    wvl = nc.dram_tensor("wvl", [128, NE, D], FP8, kind="ExternalInput")
    woh = nc.dram_tensor("woh", [128, G, E], FP8, kind="ExternalInput")
    wol = nc.dram_tensor("wol", [128, G, E], FP8, kind="ExternalInput")
    cosT = nc.dram_tensor("cosT", [D, S], BF16, kind="ExternalInput")
    sinT = nc.dram_tensor("sinT", [D, S], BF16, kind="ExternalInput")
    rotP = nc.dram_tensor("rotP", [128, 128], BF16, kind="ExternalInput")
    onesb = nc.dram_tensor("onesb", [128, 1], BF16, kind="ExternalInput")
    out = nc.dram_tensor("out", [S, E], BF16, kind="ExternalOutput")
    with tile.TileContext(nc) as tc:
        _emit(nc, tc, xh, xl, wqh, wql, wkh, wkl, wvh, wvl, woh, wol, cosT,
              sinT, rotP, onesb, out)
    nc.compile()
    return nc


def _rope_tables():
    inv = 1.0 / (ROPE_BASE ** (np.arange(0, D, 2, dtype=np.float64) / D))
    t = np.arange(S, dtype=np.float64)
    freqs = t[:, None] * inv[None, :]                    # [S, D/2]
    emb = np.concatenate([freqs, freqs], axis=-1)        # [S, D]
    cosT = np.cos(emb).T.astype(ml_dtypes.bfloat16)      # [D, S]
    sinT = np.sin(emb).T.astype(ml_dtypes.bfloat16)
    return np.ascontiguousarray(cosT), np.ascontiguousarray(sinT)


def _rot_perm():
    # rot(q)[d] = -q[d+64] for d<64, +q[d-64] for d>=64, as a stationary
    # matmul operand: rot = P^T @ q with P[k, m] below.
    p = np.zeros((128, 128), dtype=ml_dtypes.bfloat16)
    for d in range(64):
        p[d + 64, d] = -1.0
        p[d, d + 64] = 1.0
    return p


def _pm(a, nblk):
    """[K, M] -> partition-major [128, nblk, M] (K = nblk*128)."""
    k, m = a.shape
    return np.ascontiguousarray(a.reshape(nblk, 128, m).transpose(1, 0, 2))


_NC = None
LAST_RESULTS = None


def kernel(hidden_states, wq, wk, wv, wo):
    global _NC, LAST_RESULTS
    if _NC is None:
        _NC = _build()
    cosT, sinT = _rope_tables()
    onesb = np.full((128, 1), RED, dtype=ml_dtypes.bfloat16)
    rotP = _rot_perm()
    f8 = ml_dtypes.float8_e4m3

    def planes(a, scale):
        hi = (scale * a).astype(f8)
        lo = (scale * a - hi.astype(np.float32)).astype(f8)
        return hi, lo

    hs = np.asarray(hidden_states, dtype=np.float32)
    wq = np.asarray(wq, dtype=np.float32)
    wk = np.asarray(wk, dtype=np.float32)
    wv = np.asarray(wv, dtype=np.float32)
    wo = np.asarray(wo, dtype=np.float32)
    xplanes = []
    for b in range(B):
        hi, lo = planes(np.ascontiguousarray(hs[b].T), AX)
        xplanes.append((_pm(hi, NE), _pm(lo, NE)))

    in_maps = []
    for core in range(NCORES):
        b, g = divmod(core, G)
        wqh_, wql_ = planes(wq[:, GD * g:GD * (g + 1)], AW)
        wkh_, wkl_ = planes(wk[:, D * g:D * (g + 1)], AW)
        wvh_, wvl_ = planes(wv[:, D * g:D * (g + 1)], AW)
        woh_, wol_ = planes(wo[GD * g:GD * (g + 1), :], AW)
        in_maps.append({
            "xh": xplanes[b][0],
            "xl": xplanes[b][1],
            "wqh": _pm(wqh_, NE),
            "wql": _pm(wql_, NE),
            "wkh": _pm(wkh_, NE),
            "wkl": _pm(wkl_, NE),
            "wvh": _pm(wvh_, NE),
            "wvl": _pm(wvl_, NE),
            "woh": _pm(woh_, G),
            "wol": _pm(wol_, G),
            "cosT": cosT,
            "sinT": sinT,
            "rotP": rotP,
            "onesb": onesb,
        })

    res = run_bass_kernel_spmd(_NC, in_maps, list(range(NCORES)))
    LAST_RESULTS = res
    outs = [np.asarray(res.results[i]["out"], dtype=np.float32)
            for i in range(NCORES)]
    full = np.stack([sum(outs[b * G:(b + 1) * G]) for b in range(B)], axis=0)
    return (full / PSC).astype(np.float32)


# revision 13
# speedup vs baseline: 1.2283x; 1.0032x over previous
"""GQA (16 q-heads / 4 kv-heads, D=128, S=2048, E=2048, B=2) on 8 trn2 cores.

Sharding: core = 4*b + g  (b in {0,1} batch, g in {0..3} kv-head group).
Each core computes its batch's 4 query heads (one kv group) end-to-end and
the host sums the 4 partial o_proj outputs per batch.

v4 (balanced Act/PE, paired exp, direct stores):
  - Host pre-arranges all tensors partition-major so every DMA moves >=512B
    contiguous runs (no strided-transpose DMAs); wq/wk/wv/wo resident in
    SBUF fp8 hi/lo planes, x resident per chunk.
  - V projected directly into natural [keys, D] layout (x-tile stationary,
    wv moving) -- no PE transpose, no staging copy.
  - Attention in four 512-wide query chunks; key tiles processed in PAIRS:
    scores for tiles 2j,2j+1 -> one [128,2,512] PSUM tile -> ONE Act exp
    instruction (halves Act instruction overhead); AV per tile in bf16.
    Pipeline depth 2 pairs: scp(j) + drained fill units run before
    avp(j-2), so exp latency is fully hidden.
  - Deferred projection/o_proj work queued as ~2-matmul units and drained a
    few per pair-iteration: B0<-Q1, B1<-Q2+O(c0), B2<-Q3+O(c1), B3<-O(c2),
    tail<-O(c3).  o_proj accumulates in PSUM and stores PSUM->DRAM direct.
"""

import numpy as np
import ml_dtypes

import concourse.bass as bass
import concourse.bacc as bacc
import concourse.mybir as mybir
import concourse.tile as tile
from concourse.bass_utils import run_bass_kernel_spmd

B, S, E = 2, 2048, 2048
H, HKV, D = 16, 4, 128
G = H // HKV          # 4 query heads per kv group
GD = G * D            # 512 channels per group
NCORES = 8
SCALE = 1.0 / float(np.sqrt(D))
ROPE_BASE = 10000.0
AX = 16.0             # fp8 plane scale for x
AW = 64.0             # fp8 plane scale for wq/wk/wv/wo
PSC = AX * AW         # q/k/v come out scaled by PSC
SCALE_EFF = SCALE / (PSC * PSC)   # folds the q*k scale into exp
AO = 16.0             # fp8 plane scale for the normalized attention output
# the softmax reduce uses (PSC/AO)-valued "ones", so ot = AO * attn_out and
# the o_proj result comes out scaled by AO*AW = PSC; the host divides once.
RED = PSC / AO

NE = E // 128         # 16 e-blocks (contraction for projections)
NC4 = S // 512        # 4 position chunks of 512
NST = S // 128        # 16 sk-tiles of 128
NP = NST // 2         # 8 sk-tile PAIRS

F32 = mybir.dt.float32
BF16 = mybir.dt.bfloat16
FP8 = mybir.dt.float8e4
DR = mybir.MatmulPerfMode.DoubleRow
AF = mybir.ActivationFunctionType
OP = mybir.AluOpType

PLANES = ((0, 0), (0, 1), (1, 0))   # (w_plane, x_plane): HH, HL, LH


def _emit(nc, tc, xh, xl, wqh, wql, wkh, wkl, wvh, wvl, woh, wol, cosT,
          sinT, rotP, onesb, out):
    from contextlib import ExitStack
    import collections
    es = ExitStack()
    with es:
        cpool = es.enter_context(tc.tile_pool(name="const", bufs=1))
        xpool = es.enter_context(tc.tile_pool(name="xs", bufs=1))
        rpool = es.enter_context(tc.tile_pool(name="rope", bufs=2))
        etpool = es.enter_context(tc.tile_pool(name="et", bufs=4))
        bcspool = es.enter_context(tc.tile_pool(name="bcs", bufs=2))
        dnpool = es.enter_context(tc.tile_pool(name="dn", bufs=2))
        rcpool = es.enter_context(tc.tile_pool(name="rc", bufs=2))
        otpool = es.enter_context(tc.tile_pool(name="ot", bufs=2))
        ostgpool = es.enter_context(tc.tile_pool(name="ostg", bufs=6))
        pssc = es.enter_context(
            tc.tile_pool(name="pssc", bufs=2, space=bass.MemorySpace.PSUM))
        psav = es.enter_context(
            tc.tile_pool(name="psav", bufs=1, space=bass.MemorySpace.PSUM))
        psmx = es.enter_context(
            tc.tile_pool(name="psmx", bufs=3, space=bass.MemorySpace.PSUM))

        # ---- persistent SBUF tensors ----
        rp_sb = cpool.tile([128, 128], BF16, tag="rp")
        ones_sb = cpool.tile([128, 1], BF16, tag="ones")
        cos_sb = cpool.tile([D, S], BF16, tag="cos")
        sin_sb = cpool.tile([D, S], BF16, tag="sin")
        wk_t = [cpool.tile([128, NE, D], FP8, tag=f"wkt{i}", name=f"wkt{i}")
                for i in range(2)]
        wv_t = [cpool.tile([128, NE, D], FP8, tag=f"wvt{i}", name=f"wvt{i}")
                for i in range(2)]
        wq_t = [cpool.tile([128, NE, GD], FP8, tag=f"wqt{i}", name=f"wqt{i}")
                for i in range(2)]
        wo_t = [cpool.tile([128, G, E], FP8, tag=f"wot{i}", name=f"wot{i}")
                for i in range(2)]
        onescol = cpool.tile([1, 128], BF16, tag="onescol")
        kt = cpool.tile([D, S], BF16, tag="kt")
        qt = [cpool.tile([D, S], BF16, tag=f"qt{h}", name=f"qt{h}")
              for h in range(G)]
        vn = cpool.tile([128, NST, D], BF16, tag="vn")

        xt = {}

        def load_x(c4, nsplit):
            for i, t in enumerate((xh, xl)):
                xtile = xpool.tile([128, NE, 512], FP8, tag=f"x{c4}_{i}",
                                   name=f"x{c4}_{i}")
                step = NE // nsplit
                for s in range(nsplit):
                    nc.sync.dma_start(
                        out=xtile[:, s * step:(s + 1) * step, :],
                        in_=t.ap()[:, s * step:(s + 1) * step,
                                   c4 * 512:(c4 + 1) * 512])
                xt[(c4, i)] = xtile

        # ---- DMA schedule (dependency order; contiguous runs >=512B) ----
        for i, t in enumerate((wkh, wkl)):
            nc.sync.dma_start(out=wk_t[i][:], in_=t.ap())
        load_x(0, 4)
        nc.sync.dma_start(out=rp_sb[:], in_=rotP.ap())
        for i, t in enumerate((wvh, wvl)):
            nc.sync.dma_start(out=wv_t[i][:], in_=t.ap())

        def load_cs(c4):
            sl = slice(c4 * 512, (c4 + 1) * 512)
            nc.sync.dma_start(out=cos_sb[:, sl], in_=cosT.ap()[:, sl])
            nc.sync.dma_start(out=sin_sb[:, sl], in_=sinT.ap()[:, sl])

        load_cs(0)
        for i, t in enumerate((wqh, wql)):
            for sp in range(2):
                nc.sync.dma_start(
                    out=wq_t[i][:, sp * 8:(sp + 1) * 8, :],
                    in_=t.ap()[:, sp * 8:(sp + 1) * 8, :])
        load_x(1, 2)
        load_cs(1)
        load_x(2, 2)
        load_cs(2)
        load_x(3, 2)
        load_cs(3)
        nc.sync.dma_start(out=ones_sb[:], in_=onesb.ap())
        nc.vector.memset(onescol[:], 1.0)
        for i, t in enumerate((woh, wol)):
            for sp in range(2):
                nc.sync.dma_start(
                    out=wo_t[i][:, sp * 2:(sp + 1) * 2, :],
                    in_=t.ap()[:, sp * 2:(sp + 1) * 2, :])

        # ---- fill-unit queues (labelled; require() force-drains FIFO
        # until a label's units are all emitted -- keeps emission order
        # consistent with data dependencies) ----
        fill = collections.deque()
        tailq = collections.deque()
        pending = collections.Counter()

        def enq(label, fn, q=None):
            (fill if q is None else q).append((label, fn))
            pending[label] += 1

        def drain(n, q=None):
            q = fill if q is None else q
            while n > 0 and q:
                lab, fn = q.popleft()
                fn()
                pending[lab] -= 1
                n -= 1

        def require(label):
            while pending.get(label, 0) > 0:
                lab, fn = fill.popleft()
                fn()
                pending[lab] -= 1

        # ---- rope: rotate_half as signed-permutation matmul ----
        def rope_start(ps, eng):
            qraw = rpool.tile([128, 512], BF16, tag="qraw")
            if eng == 'act':
                nc.scalar.copy(qraw[:], ps[:])
            else:
                nc.vector.tensor_copy(qraw[:], ps[:])
            return qraw

        def rope_finish(dst, qraw, sl):
            tmc = rpool.tile([128, 512], BF16, tag="tmc")
            t2 = rpool.tile([128, 512], BF16, tag="t2")
            rot = psmx.tile([128, 512], F32, tag="mx", name="rot")
            nc.tensor.matmul(rot[:], rp_sb[:], qraw[:], start=True, stop=True)
            nc.gpsimd.tensor_tensor(tmc[:], qraw[:], cos_sb[:, sl], OP.mult)
            nc.vector.tensor_tensor(t2[:], rot[:], sin_sb[:, sl], OP.mult)
            nc.vector.tensor_tensor(dst, tmc[:], t2[:], OP.add)

        # ---- projections (fp8 DoubleRow, 3 quant planes) ----
        def proj_mms(wt, cslice, c4):
            mms = []
            for wi, xi in PLANES:
                for p in range(NE // 2):
                    mms.append((wt[wi][:, 2 * p:2 * p + 2, cslice],
                                xt[(c4, xi)][:, 2 * p:2 * p + 2, :]))
            return mms

        def kproj(c4):
            sl = slice(c4 * 512, (c4 + 1) * 512)
            ps = psmx.tile([128, 512], F32, tag="mx", name="ps")
            mms = proj_mms(wk_t, slice(0, D), c4)
            for i, (wa, xa) in enumerate(mms):
                nc.tensor.matmul(ps[:], wa, xa, perf_mode=DR,
                                 start=(i == 0), stop=(i == len(mms) - 1))
            return ps, sl

        def vproj_mms(c4):
            vp = psmx.tile([128, 4, 128], F32, tag="mx", name="vp")
            for i in range(4):
                ksl = slice(i * 128, (i + 1) * 128)
                j = 0
                for wi, xi in PLANES:
                    for p in range(NE // 2):
                        nc.tensor.matmul(
                            vp[:, i, :],
                            xt[(c4, xi)][:, 2 * p:2 * p + 2, ksl],
                            wv_t[wi][:, 2 * p:2 * p + 2, :],
                            perf_mode=DR, start=(j == 0), stop=(j == 23))
                        j += 1
            return vp

        def vn_copy(c4, vp, eng):
            dst = vn[:, c4 * 4:(c4 + 1) * 4, :]
            if eng == 'act':
                nc.scalar.copy(dst, vp[:])
            elif eng == 'pool':
                nc.gpsimd.tensor_copy(dst, vp[:])
            else:
                nc.vector.tensor_copy(dst, vp[:])

        def qproj(h, c4):
            # phase-A inline Q projection; rope finished by caller interleave
            sl = slice(c4 * 512, (c4 + 1) * 512)
            ps = psmx.tile([128, 512], F32, tag="mx", name="ps")
            mms = proj_mms(wq_t, slice(h * D, (h + 1) * D), c4)
            for i, (wa, xa) in enumerate(mms):
                nc.tensor.matmul(ps[:], wa, xa, perf_mode=DR,
                                 start=(i == 0), stop=(i == len(mms) - 1))
            return ps, sl

        def enqueue_qproj(h, c4):
            sl = slice(c4 * 512, (c4 + 1) * 512)
            lab = f"Q{c4}h{h}"
            state = {}
            nmm = 24

            def mk(j):
                def unit():
                    if j == 0:
                        state['ps'] = psmx.tile([128, 512], F32, tag="mx",
                                                name="ps")
                        state['mms'] = proj_mms(
                            wq_t, slice(h * D, (h + 1) * D), c4)
                    ps = state['ps']
                    for jj in (2 * j, 2 * j + 1):
                        wa, xa = state['mms'][jj]
                        nc.tensor.matmul(ps[:], wa, xa, perf_mode=DR,
                                         start=(jj == 0), stop=(jj == nmm - 1))
                return unit
            for j in range(nmm // 2):
                enq(lab, mk(j))

            def fin():
                qraw = rope_start(state['ps'], 'dve')
                drain(2)   # cover the DVE copy latency with queued PE work
                rope_finish(qt[h][:, sl], qraw, sl)
            enq(lab, fin)

        def enqueue_kproj(c4):
            sl = slice(c4 * 512, (c4 + 1) * 512)
            lab = f"K{c4}"
            state = {}
            nmm = 24

            def mk(j):
                def unit():
                    if j == 0:
                        state['ps'] = psmx.tile([128, 512], F32, tag="mx",
                                                name="ps")
                        state['mms'] = proj_mms(wk_t, slice(0, D), c4)
                    ps = state['ps']
                    for jj in (2 * j, 2 * j + 1):
                        wa, xa = state['mms'][jj]
                        nc.tensor.matmul(ps[:], wa, xa, perf_mode=DR,
                                         start=(jj == 0), stop=(jj == nmm - 1))
                return unit
            for j in range(nmm // 2):
                enq(lab, mk(j))

            def fin():
                qraw = rope_start(state['ps'], 'dve')
                drain(2)   # cover the DVE copy latency with queued PE work
                rope_finish(kt[:, sl], qraw, sl)
            enq(lab, fin)

        def enqueue_vproj(c4):
            lab = f"V{c4}"
            state = {}

            def mkmm(i, g):
                def unit():
                    if i == 0 and g == 0:
                        state['vp'] = psmx.tile([128, 4, 128], F32, tag="mx",
                                                name="vp")
                    vp = state['vp']
                    ksl = slice(i * 128, (i + 1) * 128)
                    mms = [(xt[(c4, xi)][:, 2 * p:2 * p + 2, ksl],
                            wv_t[wi][:, 2 * p:2 * p + 2, :])
                           for wi, xi in PLANES for p in range(NE // 2)]
                    for jj in range(8 * g, 8 * g + 8):
                        sa, ma = mms[jj]
                        nc.tensor.matmul(vp[:, i, :], sa, ma, perf_mode=DR,
                                         start=(jj == 0), stop=(jj == 23))
                return unit

            def mkcp(i):
                def unit():
                    nc.vector.tensor_copy(vn[:, c4 * 4 + i, :],
                                          state['vp'][:, i, :])
                return unit
            for i in range(4):
                for g in range(3):
                    enq(lab, mkmm(i, g))
                enq(lab, mkcp(i))

        # ---- o_proj: ot (fp8 hi/lo planes) @ wo, PSUM -> bf16 SBUF
        # staging (two 512-col groups share one [128,1024] staging tile and
        # one store) ----
        def enqueue_oproj(ci, oth, otl, q):
            off = ci * 512
            for st in range(4):
                ssl = slice(st * 128, (st + 1) * 128)
                shared = {}
                for eo in range(4):
                    esl = slice(eo * 512, (eo + 1) * 512)
                    state = {}
                    # head-pair i=0 mms first: they only need heads 0/1 of
                    # the ot planes, so the tail can start before the last
                    # head's epilogue lands.
                    mms = []
                    for i in range(2):
                        for src, wi in ((oth, 0), (oth, 1), (otl, 0)):
                            mms.append((src[:, 2 * i:2 * i + 2, ssl],
                                        wo_t[wi][:, 2 * i:2 * i + 2, esl]))

                    def mk(st, eo, j, mms=mms, state=state, shared=shared):
                        def unit():
                            if j == 0:
                                state['op'] = psmx.tile([128, 512], F32,
                                                        tag="mx", name="op")
                            op = state['op']
                            for jj in (2 * j, 2 * j + 1):
                                oa, wa = mms[jj]
                                nc.tensor.matmul(op[:], oa, wa, perf_mode=DR,
                                                 start=(jj == 0),
                                                 stop=(jj == 5))
                            if j == 2:
                                half = eo % 2
                                if half == 0:
                                    shared['ostg'] = ostgpool.tile(
                                        [128, 1024], BF16, tag="ostg",
                                        name="ostg")
                                ostg = shared['ostg']
                                dst = ostg[:, half * 512:(half + 1) * 512]
                                # GPSIMD cannot read PSUM: alternate Act/DVE
                                if (st + eo // 2) % 2 == 0:
                                    nc.scalar.copy(dst, op[:])
                                else:
                                    nc.vector.tensor_copy(dst, op[:])
                                if half == 1:
                                    nc.sync.dma_start(
                                        out=out.ap()[off + st * 128:
                                                     off + (st + 1) * 128,
                                                     (eo - 1) * 512:
                                                     (eo + 1) * 512],
                                        in_=ostg[:])
                        return unit
                    for j in range(3):
                        enq(f"O{ci}", mk(st, eo, j), q)

        # ---- K/V as fill units (not used in final schedule; kept simple) --

        # ---- attention: paired key tiles, depth-2 pair pipeline ----
        def attn_head(ci, h, oth, otl, drain_n):
            off = ci * 512
            qsl = slice(off, off + 512)
            require(f"Q{ci}h{h}")
            dn = dnpool.tile([128, 512], BF16, tag="dn")
            av = psav.tile([D, 512], F32, tag="av")

            def scp_exp(j):
                require(f"K{(2 * j + 1) // 4}")
                sc = pssc.tile([128, 2, 512], F32, tag="sc")
                for tt in range(2):
                    t = 2 * j + tt
                    nc.tensor.matmul(sc[:, tt, :],
                                     kt[:, t * 128:(t + 1) * 128],
                                     qt[h][:, qsl], start=True, stop=True)
                et = etpool.tile([128, 2, 512], BF16, tag="et")
                nc.scalar.activation(et[:], sc[:], AF.Exp, scale=SCALE_EFF)
                return et

            def avp(j, et):
                require(f"V{(2 * j + 1) // 4}")
                for tt in range(2):
                    t = 2 * j + tt
                    nc.tensor.matmul(av[:], vn[:, t, :], et[:, tt, :],
                                     start=(t == 0), stop=(t == NST - 1))

            def dnp(j, et):
                for tt in range(2):
                    if j == 0 and tt == 0:
                        nc.vector.tensor_copy(dn[:], et[:, 0, :])
                    else:
                        nc.vector.tensor_tensor(dn[:], dn[:], et[:, tt, :],
                                                OP.add)

            ets = {}
            ets[0] = scp_exp(0)
            ets[1] = scp_exp(1)
            drain(drain_n)
            for j in range(2, NP):
                ets[j] = scp_exp(j)
                drain(drain_n)
                avp(j - 2, ets[j - 2])
                dnp(j - 2, ets[j - 2])
                del ets[j - 2]
            drain(drain_n)
            avp(NP - 2, ets[NP - 2])
            dnp(NP - 2, ets[NP - 2])
            drain(drain_n)
            avp(NP - 1, ets[NP - 1])
            dnp(NP - 1, ets[NP - 1])
            drain(4)
            # epilogue: denominator reduce, reciprocal, normalize, fp8 planes
            sm = psmx.tile([1, 512], F32, tag="mx", name="sm")
            nc.tensor.matmul(sm[:], ones_sb[:, 0:1], dn[:],
                             start=True, stop=True)
            rc = rcpool.tile([1, 512], BF16, tag="rc")
            with nc.allow_low_precision(reason="bf16 softmax denom recip"):
                nc.vector.reciprocal(rc[:], sm[:])
            otf = rpool.tile([D, 512], F32, tag="otf")
            if ci == 3 and h == 3:
                # last head before the tail: shortest possible chain --
                # broadcast via a 1-row PE matmul, fp8-hi copy on Act.
                bcp = psmx.tile([128, 512], F32, tag="mx", name="bcp")
                nc.tensor.matmul(bcp[:], onescol[:], rc[:],
                                 start=True, stop=True)
                nc.vector.tensor_tensor(otf[:], av[:], bcp[:], OP.mult)
                nc.scalar.copy(oth[:, h, :], otf[:])
            else:
                bcs = bcspool.tile([128, 512], BF16, tag="bcs")
                nc.gpsimd.partition_broadcast(bcs[:], rc[:])
                nc.vector.tensor_tensor(otf[:], av[:], bcs[:], OP.mult)
                nc.gpsimd.tensor_copy(oth[:, h, :], otf[:])
            nc.vector.tensor_tensor(otl[:, h, :], otf[:], oth[:, h, :],
                                    OP.subtract)

        # ====== phase A: K/V chunks 0-1, Q chunk 0; rest drains into B0 =====
        ps, sl = kproj(0)
        qraw = rope_start(ps, 'act')
        vp = vproj_mms(0)
        rope_finish(kt[:, sl], qraw, sl)
        vn_copy(0, vp, 'act')
        prev = None
        for h in range(G):
            ps, sl = qproj(h, 0)
            if prev is not None:
                ph, pq, psl = prev
                rope_finish(qt[ph][:, psl], pq, psl)
            qraw = rope_start(ps, 'act')
            prev = (h, qraw, sl)
        ps, sl = kproj(1)
        ph, pq, psl = prev
        rope_finish(qt[ph][:, psl], pq, psl)
        qraw = rope_start(ps, 'act')
        vp = vproj_mms(1)
        rope_finish(kt[:, sl], qraw, sl)
        vn_copy(1, vp, 'act')

        # ================= B windows: attention + drained fills =============
        def ot_planes(ci):
            hi = otpool.tile([128, G, 512], FP8, tag="oth", name=f"oth{ci}")
            lo = otpool.tile([128, G, 512], FP8, tag="otl", name=f"otl{ci}")
            return hi, lo

        planes = {}
        # B0: fills = K/V chunks 2,3 (ledger-paced), then Q chunk 1
        planes[0] = ot_planes(0)
        enqueue_kproj(2)
        enqueue_vproj(2)
        enqueue_kproj(3)
        enqueue_vproj(3)
        for hq in range(G):
            enqueue_qproj(hq, 1)
        for h in range(G):
            attn_head(0, h, planes[0][0], planes[0][1], 6 if h == 0 else 2)
        # B1: fills += Q chunk 2 + o_proj of chunk 0
        planes[1] = ot_planes(1)
        for h in range(G):
            if h == 0:
                for hq in range(G):
                    enqueue_qproj(hq, 2)
            elif h == 1:
                enqueue_oproj(0, planes[0][0], planes[0][1], fill)
            attn_head(1, h, planes[1][0], planes[1][1], 2)
        # B2: fills += Q chunk 3 + o_proj of chunk 1
        planes[2] = ot_planes(2)
        for h in range(G):
            if h == 0:
                for hq in range(G):
                    enqueue_qproj(hq, 3)
            elif h == 1:
                enqueue_oproj(1, planes[1][0], planes[1][1], fill)
            attn_head(2, h, planes[2][0], planes[2][1], 2)
        # B3: fills += o_proj of chunk 2
        planes[3] = ot_planes(3)
        for h in range(G):
            if h == 0:
                enqueue_oproj(2, planes[2][0], planes[2][1], fill)
            attn_head(3, h, planes[3][0], planes[3][1], 2)
        drain(len(fill))
        # tail: o_proj of chunk 3
        enqueue_oproj(3, planes[3][0], planes[3][1], tailq)
        drain(len(tailq), tailq)


def _build():
    nc = bacc.Bacc("TRN2", target_bir_lowering=False, debug=False,
                   num_devices=NCORES)
    xh = nc.dram_tensor("xh", [128, NE, S], FP8, kind="ExternalInput")
    xl = nc.dram_tensor("xl", [128, NE, S], FP8, kind="ExternalInput")
    wqh = nc.dram_tensor("wqh", [128, NE, GD], FP8, kind="ExternalInput")
    wql = nc.dram_tensor("wql", [128, NE, GD], FP8, kind="ExternalInput")
    wkh = nc.dram_tensor("wkh", [128, NE, D], FP8, kind="ExternalInput")
    wkl = nc.dram_tensor("wkl", [128, NE, D], FP8, kind="ExternalInput")
    wvh = nc.dram_tensor("wvh", [128, NE, D], FP8, kind="ExternalInput")
    wvl = nc.dram_tensor("wvl", [128, NE, D], FP8, kind="ExternalInput")
    woh = nc.dram_tensor("woh", [128, G, E], FP8, kind="ExternalInput")
    wol = nc.dram_tensor("wol", [128, G, E], FP8, kind="ExternalInput")
    cosT = nc.dram_tensor("cosT", [D, S], BF16, kind="ExternalInput")
    sinT = nc.dram_tensor("sinT", [D, S], BF16, kind="ExternalInput")
    rotP = nc.dram_tensor("rotP", [128, 128], BF16, kind="ExternalInput")
    onesb = nc.dram_tensor("onesb", [128, 1], BF16, kind="ExternalInput")
    out = nc.dram_tensor("out", [S, E], BF16, kind="ExternalOutput")
    with tile.TileContext(nc) as tc:
        _emit(nc, tc, xh, xl, wqh, wql, wkh, wkl, wvh, wvl, woh, wol, cosT,
              sinT, rotP, onesb, out)
    nc.compile()
    return nc


def _rope_tables():
    inv = 1.0 / (ROPE_BASE ** (np.arange(0, D, 2, dtype=np.float64) / D))
    t = np.arange(S, dtype=np.float64)
    freqs = t[:, None] * inv[None, :]                    # [S, D/2]
    emb = np.concatenate([freqs, freqs], axis=-1)        # [S, D]
    cosT = np.cos(emb).T.astype(ml_dtypes.bfloat16)      # [D, S]
    sinT = np.sin(emb).T.astype(ml_dtypes.bfloat16)
    return np.ascontiguousarray(cosT), np.ascontiguousarray(sinT)


def _rot_perm():
    # rot(q)[d] = -q[d+64] for d<64, +q[d-64] for d>=64, as a stationary
    # matmul operand: rot = P^T @ q with P[k, m] below.
    p = np.zeros((128, 128), dtype=ml_dtypes.bfloat16)
    for d in range(64):
        p[d + 64, d] = -1.0
        p[d, d + 64] = 1.0
    return p


def _pm(a, nblk):
    """[K, M] -> partition-major [128, nblk, M] (K = nblk*128)."""
    k, m = a.shape
    return np.ascontiguousarray(a.reshape(nblk, 128, m).transpose(1, 0, 2))


_NC = None
LAST_RESULTS = None


def kernel(hidden_states, wq, wk, wv, wo):
    global _NC, LAST_RESULTS
    if _NC is None:
        _NC = _build()
    cosT, sinT = _rope_tables()
    onesb = np.full((128, 1), RED, dtype=ml_dtypes.bfloat16)
    rotP = _rot_perm()
    f8 = ml_dtypes.float8_e4m3

    def planes(a, scale):
        hi = (scale * a).astype(f8)
        lo = (scale * a - hi.astype(np.float32)).astype(f8)
        return hi, lo

    hs = np.asarray(hidden_states, dtype=np.float32)
    wq = np.asarray(wq, dtype=np.float32)
    wk = np.asarray(wk, dtype=np.float32)
    wv = np.asarray(wv, dtype=np.float32)
    wo = np.asarray(wo, dtype=np.float32)
    xplanes = []
    for b in range(B):
        hi, lo = planes(np.ascontiguousarray(hs[b].T), AX)
        xplanes.append((_pm(hi, NE), _pm(lo, NE)))

    in_maps = []
    for core in range(NCORES):
        b, g = divmod(core, G)
        wqh_, wql_ = planes(wq[:, GD * g:GD * (g + 1)], AW)
        wkh_, wkl_ = planes(wk[:, D * g:D * (g + 1)], AW)
        wvh_, wvl_ = planes(wv[:, D * g:D * (g + 1)], AW)
        woh_, wol_ = planes(wo[GD * g:GD * (g + 1), :], AW)
        in_maps.append({
            "xh": xplanes[b][0],
            "xl": xplanes[b][1],
            "wqh": _pm(wqh_, NE),
            "wql": _pm(wql_, NE),
            "wkh": _pm(wkh_, NE),
            "wkl": _pm(wkl_, NE),
            "wvh": _pm(wvh_, NE),
            "wvl": _pm(wvl_, NE),
            "woh": _pm(woh_, G),
            "wol": _pm(wol_, G),
            "cosT": cosT,
            "sinT": sinT,
            "rotP": rotP,
            "onesb": onesb,
        })

    res = run_bass_kernel_spmd(_NC, in_maps, list(range(NCORES)))
    LAST_RESULTS = res
    outs = [np.asarray(res.results[i]["out"], dtype=np.float32)
            for i in range(NCORES)]
    full = np.stack([sum(outs[b * G:(b + 1) * G]) for b in range(B)], axis=0)
    return (full / PSC).astype(np.float32)


# revision 14
# speedup vs baseline: 1.2315x; 1.0026x over previous
"""GQA (16 q-heads / 4 kv-heads, D=128, S=2048, E=2048, B=2) on 8 trn2 cores.

Sharding: core = 4*b + g  (b in {0,1} batch, g in {0..3} kv-head group).
Each core computes its batch's 4 query heads (one kv group) end-to-end and
the host sums the 4 partial o_proj outputs per batch.

v4 (balanced Act/PE, paired exp, direct stores):
  - Host pre-arranges all tensors partition-major so every DMA moves >=512B
    contiguous runs (no strided-transpose DMAs); wq/wk/wv/wo resident in
    SBUF fp8 hi/lo planes, x resident per chunk.
  - V projected directly into natural [keys, D] layout (x-tile stationary,
    wv moving) -- no PE transpose, no staging copy.
  - Attention in four 512-wide query chunks; key tiles processed in PAIRS:
    scores for tiles 2j,2j+1 -> one [128,2,512] PSUM tile -> ONE Act exp
    instruction (halves Act instruction overhead); AV per tile in bf16.
    Pipeline depth 2 pairs: scp(j) + drained fill units run before
    avp(j-2), so exp latency is fully hidden.
  - Deferred projection/o_proj work queued as ~2-matmul units and drained a
    few per pair-iteration: B0<-Q1, B1<-Q2+O(c0), B2<-Q3+O(c1), B3<-O(c2),
    tail<-O(c3).  o_proj accumulates in PSUM and stores PSUM->DRAM direct.
"""

import numpy as np
import ml_dtypes

import concourse.bass as bass
import concourse.bacc as bacc
import concourse.mybir as mybir
import concourse.tile as tile
from concourse.bass_utils import run_bass_kernel_spmd

B, S, E = 2, 2048, 2048
H, HKV, D = 16, 4, 128
G = H // HKV          # 4 query heads per kv group
GD = G * D            # 512 channels per group
NCORES = 8
SCALE = 1.0 / float(np.sqrt(D))
ROPE_BASE = 10000.0
AX = 16.0             # fp8 plane scale for x
AW = 64.0             # fp8 plane scale for wq/wk/wv/wo
PSC = AX * AW         # q/k/v come out scaled by PSC
SCALE_EFF = SCALE / (PSC * PSC)   # folds the q*k scale into exp
AO = 16.0             # fp8 plane scale for the normalized attention output
# the softmax reduce uses (PSC/AO)-valued "ones", so ot = AO * attn_out and
# the o_proj result comes out scaled by AO*AW = PSC; the host divides once.
RED = PSC / AO

NE = E // 128         # 16 e-blocks (contraction for projections)
NC4 = S // 512        # 4 position chunks of 512
NST = S // 128        # 16 sk-tiles of 128
NP = NST // 2         # 8 sk-tile PAIRS

F32 = mybir.dt.float32
BF16 = mybir.dt.bfloat16
FP8 = mybir.dt.float8e4
DR = mybir.MatmulPerfMode.DoubleRow
AF = mybir.ActivationFunctionType
OP = mybir.AluOpType

PLANES = ((0, 0), (0, 1), (1, 0))   # (w_plane, x_plane): HH, HL, LH


def _emit(nc, tc, xh, xl, wqh, wql, wkh, wkl, wvh, wvl, woh, wol, cosT,
          sinT, rotP, onesb, out):
    from contextlib import ExitStack
    import collections
    es = ExitStack()
    with es:
        cpool = es.enter_context(tc.tile_pool(name="const", bufs=1))
        xpool = es.enter_context(tc.tile_pool(name="xs", bufs=1))
        rpool = es.enter_context(tc.tile_pool(name="rope", bufs=2))
        etpool = es.enter_context(tc.tile_pool(name="et", bufs=4))
        bcspool = es.enter_context(tc.tile_pool(name="bcs", bufs=2))
        dnpool = es.enter_context(tc.tile_pool(name="dn", bufs=2))
        rcpool = es.enter_context(tc.tile_pool(name="rc", bufs=2))
        otpool = es.enter_context(tc.tile_pool(name="ot", bufs=2))
        ostgpool = es.enter_context(tc.tile_pool(name="ostg", bufs=6))
        pssc = es.enter_context(
            tc.tile_pool(name="pssc", bufs=2, space=bass.MemorySpace.PSUM))
        psav = es.enter_context(
            tc.tile_pool(name="psav", bufs=1, space=bass.MemorySpace.PSUM))
        psmx = es.enter_context(
            tc.tile_pool(name="psmx", bufs=3, space=bass.MemorySpace.PSUM))

        # ---- persistent SBUF tensors ----
        rp_sb = cpool.tile([128, 128], BF16, tag="rp")
        ones_sb = cpool.tile([128, 1], BF16, tag="ones")
        cos_sb = cpool.tile([D, S], BF16, tag="cos")
        sin_sb = cpool.tile([D, S], BF16, tag="sin")
        wk_t = [cpool.tile([128, NE, D], FP8, tag=f"wkt{i}", name=f"wkt{i}")
                for i in range(2)]
        wv_t = [cpool.tile([128, NE, D], FP8, tag=f"wvt{i}", name=f"wvt{i}")
                for i in range(2)]
        wq_t = [cpool.tile([128, NE, GD], FP8, tag=f"wqt{i}", name=f"wqt{i}")
                for i in range(2)]
        wo_t = [cpool.tile([128, G, E], FP8, tag=f"wot{i}", name=f"wot{i}")
                for i in range(2)]
        onescol = cpool.tile([1, 128], BF16, tag="onescol")
        kt = cpool.tile([D, S], BF16, tag="kt")
        qt = [cpool.tile([D, S], BF16, tag=f"qt{h}", name=f"qt{h}")
              for h in range(G)]
        vn = cpool.tile([128, NST, D], BF16, tag="vn")

        xt = {}

        def load_x(c4, nsplit):
            for i, t in enumerate((xh, xl)):
                xtile = xpool.tile([128, NE, 512], FP8, tag=f"x{c4}_{i}",
                                   name=f"x{c4}_{i}")
                step = NE // nsplit
                for s in range(nsplit):
                    nc.sync.dma_start(
                        out=xtile[:, s * step:(s + 1) * step, :],
                        in_=t.ap()[:, s * step:(s + 1) * step,
                                   c4 * 512:(c4 + 1) * 512])
                xt[(c4, i)] = xtile

        # ---- DMA schedule (dependency order; contiguous runs >=512B).
        # Startup interleave: wk halves between x0 quarters so the first
        # K-proj matmuls start as soon as possible.
        nc.sync.dma_start(out=wk_t[0][:, 0:8, :], in_=wkh.ap()[:, 0:8, :])
        xt0h = xpool.tile([128, NE, 512], FP8, tag="x0_0", name="x0_0")
        xt0l = xpool.tile([128, NE, 512], FP8, tag="x0_1", name="x0_1")
        xt[(0, 0)], xt[(0, 1)] = xt0h, xt0l
        nc.sync.dma_start(out=xt0h[:, 0:4, :], in_=xh.ap()[:, 0:4, 0:512])
        nc.sync.dma_start(out=wk_t[0][:, 8:16, :], in_=wkh.ap()[:, 8:16, :])
        nc.sync.dma_start(out=xt0h[:, 4:8, :], in_=xh.ap()[:, 4:8, 0:512])
        nc.sync.dma_start(out=xt0h[:, 8:12, :], in_=xh.ap()[:, 8:12, 0:512])
        nc.sync.dma_start(out=xt0h[:, 12:16, :], in_=xh.ap()[:, 12:16, 0:512])
        for sp in range(4):
            nc.sync.dma_start(out=xt0l[:, sp * 4:(sp + 1) * 4, :],
                              in_=xl.ap()[:, sp * 4:(sp + 1) * 4, 0:512])
        nc.sync.dma_start(out=wk_t[1][:], in_=wkl.ap())
        nc.sync.dma_start(out=rp_sb[:], in_=rotP.ap())
        for i, t in enumerate((wvh, wvl)):
            nc.sync.dma_start(out=wv_t[i][:], in_=t.ap())

        def load_cs(c4):
            sl = slice(c4 * 512, (c4 + 1) * 512)
            nc.sync.dma_start(out=cos_sb[:, sl], in_=cosT.ap()[:, sl])
            nc.sync.dma_start(out=sin_sb[:, sl], in_=sinT.ap()[:, sl])

        load_cs(0)
        for i, t in enumerate((wqh, wql)):
            for sp in range(2):
                nc.sync.dma_start(
                    out=wq_t[i][:, sp * 8:(sp + 1) * 8, :],
                    in_=t.ap()[:, sp * 8:(sp + 1) * 8, :])
        load_x(1, 2)
        load_cs(1)
        load_x(2, 2)
        load_cs(2)
        load_x(3, 2)
        load_cs(3)
        nc.sync.dma_start(out=ones_sb[:], in_=onesb.ap())
        nc.vector.memset(onescol[:], 1.0)
        for i, t in enumerate((woh, wol)):
            for sp in range(2):
                nc.sync.dma_start(
                    out=wo_t[i][:, sp * 2:(sp + 1) * 2, :],
                    in_=t.ap()[:, sp * 2:(sp + 1) * 2, :])

        # ---- fill-unit queues (labelled; require() force-drains FIFO
        # until a label's units are all emitted -- keeps emission order
        # consistent with data dependencies) ----
        fill = collections.deque()
        tailq = collections.deque()
        pending = collections.Counter()

        def enq(label, fn, q=None):
            (fill if q is None else q).append((label, fn))
            pending[label] += 1

        def drain(n, q=None):
            q = fill if q is None else q
            while n > 0 and q:
                lab, fn = q.popleft()
                fn()
                pending[lab] -= 1
                n -= 1

        def require(label):
            while pending.get(label, 0) > 0:
                lab, fn = fill.popleft()
                fn()
                pending[lab] -= 1

        # ---- rope: rotate_half as signed-permutation matmul ----
        def rope_start(ps, eng):
            qraw = rpool.tile([128, 512], BF16, tag="qraw")
            if eng == 'act':
                nc.scalar.copy(qraw[:], ps[:])
            else:
                nc.vector.tensor_copy(qraw[:], ps[:])
            return qraw

        def rope_finish(dst, qraw, sl):
            tmc = rpool.tile([128, 512], BF16, tag="tmc")
            t2 = rpool.tile([128, 512], BF16, tag="t2")
            rot = psmx.tile([128, 512], F32, tag="mx", name="rot")
            nc.tensor.matmul(rot[:], rp_sb[:], qraw[:], start=True, stop=True)
            nc.gpsimd.tensor_tensor(tmc[:], qraw[:], cos_sb[:, sl], OP.mult)
            nc.vector.tensor_tensor(t2[:], rot[:], sin_sb[:, sl], OP.mult)
            nc.vector.tensor_tensor(dst, tmc[:], t2[:], OP.add)

        # ---- projections (fp8 DoubleRow, 3 quant planes) ----
        def proj_mms(wt, cslice, c4):
            mms = []
            for wi, xi in PLANES:
                for p in range(NE // 2):
                    mms.append((wt[wi][:, 2 * p:2 * p + 2, cslice],
                                xt[(c4, xi)][:, 2 * p:2 * p + 2, :]))
            return mms

        def kproj(c4):
            sl = slice(c4 * 512, (c4 + 1) * 512)
            ps = psmx.tile([128, 512], F32, tag="mx", name="ps")
            mms = proj_mms(wk_t, slice(0, D), c4)
            for i, (wa, xa) in enumerate(mms):
                nc.tensor.matmul(ps[:], wa, xa, perf_mode=DR,
                                 start=(i == 0), stop=(i == len(mms) - 1))
            return ps, sl

        def vproj_mms(c4):
            vp = psmx.tile([128, 4, 128], F32, tag="mx", name="vp")
            for i in range(4):
                ksl = slice(i * 128, (i + 1) * 128)
                j = 0
                for wi, xi in PLANES:
                    for p in range(NE // 2):
                        nc.tensor.matmul(
                            vp[:, i, :],
                            xt[(c4, xi)][:, 2 * p:2 * p + 2, ksl],
                            wv_t[wi][:, 2 * p:2 * p + 2, :],
                            perf_mode=DR, start=(j == 0), stop=(j == 23))
                        j += 1
            return vp

        def vn_copy(c4, vp, eng):
            dst = vn[:, c4 * 4:(c4 + 1) * 4, :]
            if eng == 'act':
                nc.scalar.copy(dst, vp[:])
            elif eng == 'pool':
                nc.gpsimd.tensor_copy(dst, vp[:])
            else:
                nc.vector.tensor_copy(dst, vp[:])

        def qproj(h, c4):
            # phase-A inline Q projection; rope finished by caller interleave
            sl = slice(c4 * 512, (c4 + 1) * 512)
            ps = psmx.tile([128, 512], F32, tag="mx", name="ps")
            mms = proj_mms(wq_t, slice(h * D, (h + 1) * D), c4)
            for i, (wa, xa) in enumerate(mms):
                nc.tensor.matmul(ps[:], wa, xa, perf_mode=DR,
                                 start=(i == 0), stop=(i == len(mms) - 1))
            return ps, sl

        def enqueue_qproj(h, c4):
            sl = slice(c4 * 512, (c4 + 1) * 512)
            lab = f"Q{c4}h{h}"
            state = {}
            nmm = 24

            def mk(j):
                def unit():
                    if j == 0:
                        state['ps'] = psmx.tile([128, 512], F32, tag="mx",
                                                name="ps")
                        state['mms'] = proj_mms(
                            wq_t, slice(h * D, (h + 1) * D), c4)
                    ps = state['ps']
                    for jj in (2 * j, 2 * j + 1):
                        wa, xa = state['mms'][jj]
                        nc.tensor.matmul(ps[:], wa, xa, perf_mode=DR,
                                         start=(jj == 0), stop=(jj == nmm - 1))
                return unit
            for j in range(nmm // 2):
                enq(lab, mk(j))

            def fin():
                qraw = rope_start(state['ps'], 'dve')
                drain(2)   # cover the DVE copy latency with queued PE work
                rope_finish(qt[h][:, sl], qraw, sl)
            enq(lab, fin)

        def enqueue_kproj(c4):
            sl = slice(c4 * 512, (c4 + 1) * 512)
            lab = f"K{c4}"
            state = {}
            nmm = 24

            def mk(j):
                def unit():
                    if j == 0:
                        state['ps'] = psmx.tile([128, 512], F32, tag="mx",
                                                name="ps")
                        state['mms'] = proj_mms(wk_t, slice(0, D), c4)
                    ps = state['ps']
                    for jj in (2 * j, 2 * j + 1):
                        wa, xa = state['mms'][jj]
                        nc.tensor.matmul(ps[:], wa, xa, perf_mode=DR,
                                         start=(jj == 0), stop=(jj == nmm - 1))
                return unit
            for j in range(nmm // 2):
                enq(lab, mk(j))

            def fin():
                qraw = rope_start(state['ps'], 'dve')
                drain(2)   # cover the DVE copy latency with queued PE work
                rope_finish(kt[:, sl], qraw, sl)
            enq(lab, fin)

        def enqueue_vproj(c4):
            lab = f"V{c4}"
            state = {}

            def mkmm(i, g):
                def unit():
                    if i == 0 and g == 0:
                        state['vp'] = psmx.tile([128, 4, 128], F32, tag="mx",
                                                name="vp")
                    vp = state['vp']
                    ksl = slice(i * 128, (i + 1) * 128)
                    mms = [(xt[(c4, xi)][:, 2 * p:2 * p + 2, ksl],
                            wv_t[wi][:, 2 * p:2 * p + 2, :])
                           for wi, xi in PLANES for p in range(NE // 2)]
                    for jj in range(8 * g, 8 * g + 8):
                        sa, ma = mms[jj]
                        nc.tensor.matmul(vp[:, i, :], sa, ma, perf_mode=DR,
                                         start=(jj == 0), stop=(jj == 23))
                return unit

            def mkcp(i):
                def unit():
                    nc.vector.tensor_copy(vn[:, c4 * 4 + i, :],
                                          state['vp'][:, i, :])
                return unit
            for i in range(4):
                for g in range(3):
                    enq(lab, mkmm(i, g))
                enq(lab, mkcp(i))

        # ---- o_proj: ot (fp8 hi/lo planes) @ wo, PSUM -> bf16 SBUF
        # staging (two 512-col groups share one [128,1024] staging tile and
        # one store) ----
        def enqueue_oproj(ci, oth, otl, q):
            off = ci * 512
            for st in range(4):
                ssl = slice(st * 128, (st + 1) * 128)
                shared = {}
                for eo in range(4):
                    esl = slice(eo * 512, (eo + 1) * 512)
                    state = {}
                    # head-pair i=0 mms first: they only need heads 0/1 of
                    # the ot planes, so the tail can start before the last
                    # head's epilogue lands.
                    mms = []
                    for i in range(2):
                        for src, wi in ((oth, 0), (oth, 1), (otl, 0)):
                            mms.append((src[:, 2 * i:2 * i + 2, ssl],
                                        wo_t[wi][:, 2 * i:2 * i + 2, esl]))

                    def mk(st, eo, j, mms=mms, state=state, shared=shared):
                        def unit():
                            if j == 0:
                                state['op'] = psmx.tile([128, 512], F32,
                                                        tag="mx", name="op")
                            op = state['op']
                            for jj in (2 * j, 2 * j + 1):
                                oa, wa = mms[jj]
                                nc.tensor.matmul(op[:], oa, wa, perf_mode=DR,
                                                 start=(jj == 0),
                                                 stop=(jj == 5))
                            if j == 2:
                                half = eo % 2
                                if half == 0:
                                    shared['ostg'] = ostgpool.tile(
                                        [128, 1024], BF16, tag="ostg",
                                        name="ostg")
                                ostg = shared['ostg']
                                dst = ostg[:, half * 512:(half + 1) * 512]
                                # GPSIMD cannot read PSUM: alternate Act/DVE
                                if (st + eo // 2) % 2 == 0:
                                    nc.scalar.copy(dst, op[:])
                                else:
                                    nc.vector.tensor_copy(dst, op[:])
                                if half == 1:
                                    nc.sync.dma_start(
                                        out=out.ap()[off + st * 128:
                                                     off + (st + 1) * 128,
                                                     (eo - 1) * 512:
                                                     (eo + 1) * 512],
                                        in_=ostg[:])
                        return unit
                    for j in range(3):
                        enq(f"O{ci}", mk(st, eo, j), q)

        # ---- K/V as fill units (not used in final schedule; kept simple) --

        # ---- attention: paired key tiles, depth-2 pair pipeline ----
        def attn_head(ci, h, oth, otl, drain_n):
            off = ci * 512
            qsl = slice(off, off + 512)
            require(f"Q{ci}h{h}")
            dn = dnpool.tile([128, 512], BF16, tag="dn")
            av = psav.tile([D, 512], F32, tag="av")

            def scp_exp(j):
                require(f"K{(2 * j + 1) // 4}")
                sc = pssc.tile([128, 2, 512], F32, tag="sc")
                for tt in range(2):
                    t = 2 * j + tt
                    nc.tensor.matmul(sc[:, tt, :],
                                     kt[:, t * 128:(t + 1) * 128],
                                     qt[h][:, qsl], start=True, stop=True)
                et = etpool.tile([128, 2, 512], BF16, tag="et")
                nc.scalar.activation(et[:], sc[:], AF.Exp, scale=SCALE_EFF)
                return et

            def avp(j, et):
                require(f"V{(2 * j + 1) // 4}")
                for tt in range(2):
                    t = 2 * j + tt
                    nc.tensor.matmul(av[:], vn[:, t, :], et[:, tt, :],
                                     start=(t == 0), stop=(t == NST - 1))

            def dnp(j, et):
                for tt in range(2):
                    if j == 0 and tt == 0:
                        nc.vector.tensor_copy(dn[:], et[:, 0, :])
                    else:
                        nc.vector.tensor_tensor(dn[:], dn[:], et[:, tt, :],
                                                OP.add)

            ets = {}
            ets[0] = scp_exp(0)
            ets[1] = scp_exp(1)
            drain(drain_n)
            for j in range(2, NP):
                ets[j] = scp_exp(j)
                drain(drain_n if j % 2 else max(drain_n - 1, 1))
                avp(j - 2, ets[j - 2])
                dnp(j - 2, ets[j - 2])
                del ets[j - 2]
            drain(drain_n)
            avp(NP - 2, ets[NP - 2])
            dnp(NP - 2, ets[NP - 2])
            drain(drain_n)
            avp(NP - 1, ets[NP - 1])
            dnp(NP - 1, ets[NP - 1])
            drain(6)
            # epilogue: denominator reduce, reciprocal, normalize, fp8 planes
            sm = psmx.tile([1, 512], F32, tag="mx", name="sm")
            nc.tensor.matmul(sm[:], ones_sb[:, 0:1], dn[:],
                             start=True, stop=True)
            rc = rcpool.tile([1, 512], BF16, tag="rc")
            with nc.allow_low_precision(reason="bf16 softmax denom recip"):
                nc.vector.reciprocal(rc[:], sm[:])
            otf = rpool.tile([D, 512], F32, tag="otf")
            if ci == 3 and h == 3:
                # last head before the tail: shortest possible chain --
                # broadcast via a 1-row PE matmul, fp8-hi copy on Act.
                bcp = psmx.tile([128, 512], F32, tag="mx", name="bcp")
                nc.tensor.matmul(bcp[:], onescol[:], rc[:],
                                 start=True, stop=True)
                nc.vector.tensor_tensor(otf[:], av[:], bcp[:], OP.mult)
                nc.scalar.copy(oth[:, h, :], otf[:])
            else:
                bcs = bcspool.tile([128, 512], BF16, tag="bcs")
                nc.gpsimd.partition_broadcast(bcs[:], rc[:])
                nc.vector.tensor_tensor(otf[:], av[:], bcs[:], OP.mult)
                nc.gpsimd.tensor_copy(oth[:, h, :], otf[:])
            nc.vector.tensor_tensor(otl[:, h, :], otf[:], oth[:, h, :],
                                    OP.subtract)

        # ====== phase A: K/V chunks 0-1, Q chunk 0; rest drains into B0 =====
        ps, sl = kproj(0)
        qraw = rope_start(ps, 'act')
        vp = vproj_mms(0)
        rope_finish(kt[:, sl], qraw, sl)
        vn_copy(0, vp, 'act')
        prev = None
        for h in range(G):
            ps, sl = qproj(h, 0)
            if prev is not None:
                ph, pq, psl = prev
                rope_finish(qt[ph][:, psl], pq, psl)
            qraw = rope_start(ps, 'act')
            prev = (h, qraw, sl)
        ps, sl = kproj(1)
        ph, pq, psl = prev
        rope_finish(qt[ph][:, psl], pq, psl)
        qraw = rope_start(ps, 'act')
        vp = vproj_mms(1)
        rope_finish(kt[:, sl], qraw, sl)
        vn_copy(1, vp, 'act')

        # ================= B windows: attention + drained fills =============
        def ot_planes(ci):
            hi = otpool.tile([128, G, 512], FP8, tag="oth", name=f"oth{ci}")
            lo = otpool.tile([128, G, 512], FP8, tag="otl", name=f"otl{ci}")
            return hi, lo

        planes = {}
        # B0: fills = K/V chunks 2,3 (ledger-paced), then Q chunk 1
        planes[0] = ot_planes(0)
        enqueue_kproj(2)
        enqueue_vproj(2)
        enqueue_kproj(3)
        enqueue_vproj(3)
        for hq in range(G):
            enqueue_qproj(hq, 1)
        for h in range(G):
            attn_head(0, h, planes[0][0], planes[0][1], 6 if h == 0 else 2)
        # B1: fills += Q chunk 2 + o_proj of chunk 0
        planes[1] = ot_planes(1)
        for h in range(G):
            if h == 0:
                for hq in range(G):
                    enqueue_qproj(hq, 2)
            elif h == 1:
                enqueue_oproj(0, planes[0][0], planes[0][1], fill)
            attn_head(1, h, planes[1][0], planes[1][1], 2)
        # B2: fills += Q chunk 3 + o_proj of chunk 1
        planes[2] = ot_planes(2)
        for h in range(G):
            if h == 0:
                for hq in range(G):
                    enqueue_qproj(hq, 3)
            elif h == 1:
                enqueue_oproj(1, planes[1][0], planes[1][1], fill)
            attn_head(2, h, planes[2][0], planes[2][1], 2)
        # B3: fills += o_proj of chunk 2
        planes[3] = ot_planes(3)
        for h in range(G):
            if h == 0:
                enqueue_oproj(2, planes[2][0], planes[2][1], fill)
            attn_head(3, h, planes[3][0], planes[3][1], 2)
        drain(len(fill))
        # tail: o_proj of chunk 3
        enqueue_oproj(3, planes[3][0], planes[3][1], tailq)
        drain(len(tailq), tailq)


def _build():
    nc = bacc.Bacc("TRN2", target_bir_lowering=False, debug=False,
                   num_devices=NCORES)
    xh = nc.dram_tensor("xh", [128, NE, S], FP8, kind="ExternalInput")
    xl = nc.dram_tensor("xl", [128, NE, S], FP8, kind="ExternalInput")
    wqh = nc.dram_tensor("wqh", [128, NE, GD], FP8, kind="ExternalInput")
    wql = nc.dram_tensor("wql", [128, NE, GD], FP8, kind="ExternalInput")
    wkh = nc.dram_tensor("wkh", [128, NE, D], FP8, kind="ExternalInput")
    wkl = nc.dram_tensor("wkl", [128, NE, D], FP8, kind="ExternalInput")
    wvh = nc.dram_tensor("wvh", [128, NE, D], FP8, kind="ExternalInput")
    wvl = nc.dram_tensor("wvl", [128, NE, D], FP8, kind="ExternalInput")
    woh = nc.dram_tensor("woh", [128, G, E], FP8, kind="ExternalInput")
    wol = nc.dram_tensor("wol", [128, G, E], FP8, kind="ExternalInput")
    cosT = nc.dram_tensor("cosT", [D, S], BF16, kind="ExternalInput")
    sinT = nc.dram_tensor("sinT", [D, S], BF16, kind="ExternalInput")
    rotP = nc.dram_tensor("rotP", [128, 128], BF16, kind="ExternalInput")
    onesb = nc.dram_tensor("onesb", [128, 1], BF16, kind="ExternalInput")
    out = nc.dram_tensor("out", [S, E], BF16, kind="ExternalOutput")
    with tile.TileContext(nc) as tc:
        _emit(nc, tc, xh, xl, wqh, wql, wkh, wkl, wvh, wvl, woh, wol, cosT,
              sinT, rotP, onesb, out)
    nc.compile()
    return nc


def _rope_tables():
    inv = 1.0 / (ROPE_BASE ** (np.arange(0, D, 2, dtype=np.float64) / D))
    t = np.arange(S, dtype=np.float64)
    freqs = t[:, None] * inv[None, :]                    # [S, D/2]
    emb = np.concatenate([freqs, freqs], axis=-1)        # [S, D]
    cosT = np.cos(emb).T.astype(ml_dtypes.bfloat16)      # [D, S]
    sinT = np.sin(emb).T.astype(ml_dtypes.bfloat16)
    return np.ascontiguousarray(cosT), np.ascontiguousarray(sinT)


def _rot_perm():
    # rot(q)[d] = -q[d+64] for d<64, +q[d-64] for d>=64, as a stationary
    # matmul operand: rot = P^T @ q with P[k, m] below.
    p = np.zeros((128, 128), dtype=ml_dtypes.bfloat16)
    for d in range(64):
        p[d + 64, d] = -1.0
        p[d, d + 64] = 1.0
    return p


def _pm(a, nblk):
    """[K, M] -> partition-major [128, nblk, M] (K = nblk*128)."""
    k, m = a.shape
    return np.ascontiguousarray(a.reshape(nblk, 128, m).transpose(1, 0, 2))


_NC = None
LAST_RESULTS = None


def kernel(hidden_states, wq, wk, wv, wo):
    global _NC, LAST_RESULTS
    if _NC is None:
        _NC = _build()
    cosT, sinT = _rope_tables()
    onesb = np.full((128, 1), RED, dtype=ml_dtypes.bfloat16)
    rotP = _rot_perm()
    f8 = ml_dtypes.float8_e4m3

    def planes(a, scale):
        hi = (scale * a).astype(f8)
        lo = (scale * a - hi.astype(np.float32)).astype(f8)
        return hi, lo

    hs = np.asarray(hidden_states, dtype=np.float32)
    wq = np.asarray(wq, dtype=np.float32)
    wk = np.asarray(wk, dtype=np.float32)
    wv = np.asarray(wv, dtype=np.float32)
    wo = np.asarray(wo, dtype=np.float32)
    xplanes = []
    for b in range(B):
        hi, lo = planes(np.ascontiguousarray(hs[b].T), AX)
        xplanes.append((_pm(hi, NE), _pm(lo, NE)))

    in_maps = []
    for core in range(NCORES):
        b, g = divmod(core, G)
        wqh_, wql_ = planes(wq[:, GD * g:GD * (g + 1)], AW)
        wkh_, wkl_ = planes(wk[:, D * g:D * (g + 1)], AW)
        wvh_, wvl_ = planes(wv[:, D * g:D * (g + 1)], AW)
        woh_, wol_ = planes(wo[GD * g:GD * (g + 1), :], AW)
        in_maps.append({
            "xh": xplanes[b][0],
            "xl": xplanes[b][1],
            "wqh": _pm(wqh_, NE),
            "wql": _pm(wql_, NE),
            "wkh": _pm(wkh_, NE),
            "wkl": _pm(wkl_, NE),
            "wvh": _pm(wvh_, NE),
            "wvl": _pm(wvl_, NE),
            "woh": _pm(woh_, G),
            "wol": _pm(wol_, G),
            "cosT": cosT,
            "sinT": sinT,
            "rotP": rotP,
            "onesb": onesb,
        })

    res = run_bass_kernel_spmd(_NC, in_maps, list(range(NCORES)))
    LAST_RESULTS = res
    outs = [np.asarray(res.results[i]["out"], dtype=np.float32)
            for i in range(NCORES)]
    full = np.stack([sum(outs[b * G:(b + 1) * G]) for b in range(B)], axis=0)
    return (full / PSC).astype(np.float32)
